# revision 1
# baseline (speedup 1.0000x reference)
"""CRPS loss kernel for Trainium2, 8 NeuronCores (SPMD data-parallel).

reference semantics:
    p, t = prediction.ravel(), target.ravel()       # N = 16,611,840 each
    lo, hi = min(min p, min t), max(max p, max t)
    x = linspace(lo, hi, 1000)  (f32)
    cdf_q(x_i) = #{v in q : v <= x_i} / N
    return trapz(|cdf_p - cdf_t|^2, x)

Device work (per core, 1/8 shard of each tensor):
  kernel A: running min/max reduce  -> per-core (min, -max)
  kernel B: per element j = ceil((v-lo)/dx) via round-to-nearest-even cast
            (j = rint(v*A + B), A = 1/dx, B = -lo*A + 0.5), split j = 32*a+b,
            build bin-major bf16 one-hots with 64 tensor_scalar(is_equal) ops,
            accumulate joint histogram M[32,32] = sum_e onehot32(a) x onehot32(b)
            via one PE matmul per 128-element group into PSUM.
Host: combine 8 cores' histograms, fold j>=999 into bin 999, cumsum -> exact
      searchsorted counts at every x_i, then the 1000-point trapz in f64.

Shards are padded with the shard's first element to [128, 16384]; the host
subtracts the pad count from the padded value's bin (exact, same f32 math).
"""

import numpy as np
from concourse import bacc, mybir, tile
from concourse.bass_utils import run_bass_kernel_spmd

P = 128
NCORES = 8
TOTAL = 16 * 1 * 721 * 1440          # 16,611,840
SHARD = TOTAL // NCORES              # 2,076,480
KTOT = 16384                         # padded columns/core/tensor (P*KTOT = 2,097,152)
PADN = P * KTOT - SHARD              # 20,672
NB = 32                              # 32x32 = 1024 bins
NX = 1000
CHUNK = 512
NCHUNK = KTOT // CHUNK               # 32
PACK = 1                             # element-groups packed per matmul
RED_CHUNK = 2048
F32 = mybir.dt.float32
I32 = mybir.dt.int32
BF16 = mybir.dt.bfloat16
ALU = mybir.AluOpType


def _build_minmax():
    nc = bacc.Bacc()
    ins = [
        nc.declare_dram_parameter("pv", [P, KTOT], F32, isOutput=False),
        nc.declare_dram_parameter("tv", [P, KTOT], F32, isOutput=False),
    ]
    out = nc.declare_dram_parameter("mm", [1, 2], F32, isOutput=True)  # (-min, max)

    with tile.TileContext(nc) as tc:
        with (
            tc.tile_pool(name="sbuf", bufs=4) as pool,
            tc.tile_pool(name="acc", bufs=1) as apool,
        ):
            nred = (KTOT // RED_CHUNK) * 2
            mins = apool.tile([P, nred], F32)
            maxs = apool.tile([P, nred], F32)
            col = 0
            for src in ins:
                for ci in range(KTOT // RED_CHUNK):
                    v = pool.tile([P, RED_CHUNK], F32, tag="v")
                    nc.sync.dma_start(v[:], src[:, ci * RED_CHUNK:(ci + 1) * RED_CHUNK])
                    nc.vector.tensor_reduce(
                        mins[:, col:col + 1], v[:], mybir.AxisListType.X, ALU.min)
                    nc.vector.tensor_reduce(
                        maxs[:, col:col + 1], v[:], mybir.AxisListType.X, ALU.max)
                    col += 1
            pmin = apool.tile([P, 1], F32)
            pmax = apool.tile([P, 1], F32)
            nc.vector.tensor_reduce(pmin[:], mins[:], mybir.AxisListType.X, ALU.min)
            nc.vector.tensor_reduce(pmax[:], maxs[:], mybir.AxisListType.X, ALU.max)
            # cross-lane reduce only supports add/average/max -> store (-min, max)
            both = apool.tile([P, 2], F32)
            nc.vector.tensor_scalar(out=both[:, 0:1], in0=pmin[:], scalar1=-1.0,
                                    scalar2=None, op0=ALU.mult)
            nc.vector.tensor_copy(out=both[:, 1:2], in_=pmax[:])
            red = apool.tile([1, 2], F32)
            nc.gpsimd.tensor_reduce(red[:], both[:], mybir.AxisListType.C, ALU.max)
            nc.sync.dma_start(out[:], red[:])
    nc.compile()
    return nc


def _build_hist():
    nc = bacc.Bacc()
    ins = [
        nc.declare_dram_parameter("pv", [P, KTOT], F32, isOutput=False),
        nc.declare_dram_parameter("tv", [P, KTOT], F32, isOutput=False),
    ]
    ab_in = nc.declare_dram_parameter("ab", [P, 2], F32, isOutput=False)
    # hist[a, t*NB + b]: t in {0: prediction, 1: target}
    out = nc.declare_dram_parameter("hist", [NB, 2 * NB], F32, isOutput=True)

    with tile.TileContext(nc) as tc:
        with (
            tc.tile_pool(name="sbuf", bufs=3) as pool,
            tc.tile_pool(name="oh", bufs=2) as ohpool,
            tc.tile_pool(name="const", bufs=1) as cpool,
            tc.tile_pool(name="acc", bufs=1) as apool,
            tc.tile_pool(name="psum", bufs=4, space="PSUM") as psum_pool,
        ):
            ab_raw = cpool.tile([P, 2], F32)
            nc.sync.dma_start(ab_raw[:], ab_in[:])
            # DVE-bounce so tensor_scalar consumers dep on a same-engine producer
            ab = cpool.tile([P, 2], F32)
            nc.vector.tensor_copy(out=ab[:], in_=ab_raw[:])

            hacc = apool.tile([NB, 2 * NB], F32)
            nc.vector.memset(hacc[:], 0.0)

            # drains deferred one chunk so the DVE's psum->hacc add never
            # blocks the next chunk's one-hot build on the critical path
            pending = []  # (ti, m_psum)

            def drain_pending():
                while pending:
                    pti, pm = pending.pop(0)
                    # pm is [2*NB, 2*NB]; diagonal NBxNB blocks are the two
                    # packed groups' histograms
                    for blk in range(PACK):
                        nc.vector.tensor_tensor(
                            out=hacc[:, pti * NB:(pti + 1) * NB],
                            in0=hacc[:, pti * NB:(pti + 1) * NB],
                            in1=pm[blk * NB:(blk + 1) * NB, blk * NB:(blk + 1) * NB],
                            op=ALU.add,
                        )

            for ti, src in enumerate(ins):
                for ci in range(NCHUNK):
                    v = pool.tile([P, CHUNK], F32, tag="v")
                    nc.sync.dma_start(v[:], src[:, ci * CHUNK:(ci + 1) * CHUNK])
                    t1 = pool.tile([P, CHUNK], F32, tag="t1")
                    nc.vector.tensor_scalar(out=t1[:], in0=v[:], scalar1=ab[:, 0:1],
                                            scalar2=None, op0=ALU.mult)
                    zf = pool.tile([P, CHUNK], F32, tag="zf")
                    nc.vector.tensor_scalar(out=zf[:], in0=t1[:], scalar1=ab[:, 1:2],
                                            scalar2=None, op0=ALU.add)
                    ji = pool.tile([P, CHUNK], I32, tag="ji")
                    nc.vector.tensor_copy(out=ji[:], in_=zf[:])  # rint (round-even)
                    ai32 = pool.tile([P, CHUNK], I32, tag="ai32")
                    nc.vector.tensor_scalar(out=ai32[:], in0=ji[:], scalar1=5,
                                            scalar2=None, op0=ALU.arith_shift_right)
                    bi32 = pool.tile([P, CHUNK], I32, tag="bi32")
                    nc.vector.tensor_scalar(out=bi32[:], in0=ji[:], scalar1=31,
                                            scalar2=None, op0=ALU.bitwise_and)
                    # bf16 digits (0..31 exact): 16-bit in/out is_equal packs
                    ai = pool.tile([P, CHUNK], BF16, tag="ai")
                    nc.vector.tensor_copy(out=ai[:], in_=ai32[:])
                    bi = pool.tile([P, CHUNK], BF16, tag="bi")
                    nc.vector.tensor_copy(out=bi[:], in_=bi32[:])
                    # bin-major one-hots: oh[p, q*CHUNK + e] = (idx[p,e] == q)
                    oh_a = ohpool.tile([P, NB * CHUNK], BF16, tag="oh_a")
                    oh_b = ohpool.tile([P, NB * CHUNK], BF16, tag="oh_b")
                    for q in range(NB):
                        nc.vector.tensor_scalar(
                            out=oh_a[:, q * CHUNK:(q + 1) * CHUNK], in0=ai[:],
                            scalar1=float(q), scalar2=None, op0=ALU.is_equal)
                        nc.vector.tensor_scalar(
                            out=oh_b[:, q * CHUNK:(q + 1) * CHUNK], in0=bi[:],
                            scalar1=float(q), scalar2=None, op0=ALU.is_equal)
                    # PACK adjacent element-groups per matmul: lhsT/rhs
                    # [128, PACK*NB] via 3D AP [[1,PACK],[CHUNK,NB]]; out
                    # [PACK*NB, PACK*NB], diag blocks = per-group joint hists
                    oh_a4 = oh_a[:].rearrange("p (q e g) -> p e g q", q=NB, g=PACK)
                    oh_b4 = oh_b[:].rearrange("p (q e g) -> p e g q", q=NB, g=PACK)
                    m_psum = psum_pool.tile([PACK * NB, PACK * NB], F32,
                                            space="PSUM", tag="m")
                    ngrp = CHUNK // PACK
                    for e in range(ngrp):
                        nc.tensor.matmul(
                            m_psum[:],
                            lhsT=oh_a4[:, e],
                            rhs=oh_b4[:, e],
                            start=(e == 0),
                            stop=(e == ngrp - 1),
                        )
                    drain_pending()
                    pending.append((ti, m_psum))
            drain_pending()
            nc.sync.dma_start(out[:], hacc[:])
    nc.compile()
    return nc


_KERNELS = {}


def _get_kernels():
    if "mm" not in _KERNELS:
        _KERNELS["mm"] = _build_minmax()
        _KERNELS["hist"] = _build_hist()
    return _KERNELS["mm"], _KERNELS["hist"]


def _shard(flat):
    """Split [TOTAL] -> per-core padded [P, KTOT] tiles + pad values."""
    tiles, pads = [], []
    for c in range(NCORES):
        s = flat[c * SHARD:(c + 1) * SHARD]
        v0 = s[0]
        t = np.concatenate([s, np.full(PADN, v0, s.dtype)]).reshape(P, KTOT)
        tiles.append(t)
        pads.append(v0)
    return tiles, pads


def _bin_of(v, A, B):
    """Replicate device binning for a scalar f32 value."""
    t1 = np.float32(np.float32(v) * A)
    z = np.float32(t1 + B)
    j = int(np.rint(np.float64(z)))
    return min(max(j, 0), NB * NB - 1)


def kernel(prediction, target):
    nc_mm, nc_hist = _get_kernels()
    p = np.ascontiguousarray(np.asarray(prediction, dtype=np.float32).ravel())
    t = np.ascontiguousarray(np.asarray(target, dtype=np.float32).ravel())
    p_tiles, p_pads = _shard(p)
    t_tiles, t_pads = _shard(t)
    core_ids = list(range(NCORES))

    in_maps = [{"pv": p_tiles[c], "tv": t_tiles[c]} for c in core_ids]
    res = run_bass_kernel_spmd(nc_mm, in_maps, core_ids).results
    mm = np.stack([r["mm"][0] for r in res])        # [8, 2] = (-min, max)
    lo = np.float32(-(mm[:, 0].max()))
    hi = np.float32(mm[:, 1].max())

    dx = np.float32((hi - lo) / np.float32(NX - 1))
    A = np.float32(np.float32(1.0) / dx)
    B = np.float32(np.float32(-lo * A) + np.float32(0.5))
    ab = np.stack([np.full(P, A, np.float32), np.full(P, B, np.float32)], axis=1)

    in_maps = [{"pv": p_tiles[c], "tv": t_tiles[c], "ab": ab} for c in core_ids]
    res = run_bass_kernel_spmd(nc_hist, in_maps, core_ids).results

    hp = np.zeros(NB * NB, np.float64)
    ht = np.zeros(NB * NB, np.float64)
    for c in core_ids:
        h = res[c]["hist"].astype(np.float64)      # [NB, 2*NB]
        hp += h[:, :NB].ravel()
        ht += h[:, NB:].ravel()
        hp[_bin_of(p_pads[c], A, B)] -= PADN
        ht[_bin_of(t_pads[c], A, B)] -= PADN

    # fold j >= NX-1 into bin NX-1, cumsum -> counts at x_i
    hp[NX - 1] += hp[NX:].sum()
    ht[NX - 1] += ht[NX:].sum()
    cnt_p = np.cumsum(hp[:NX])
    cnt_t = np.cumsum(ht[:NX])

    n = np.float64(TOTAL)
    diff = np.abs(cnt_p / n - cnt_t / n)
    y = diff * diff
    x = np.linspace(np.float64(lo), np.float64(hi), NX)
    dxs = x[1:] - x[:-1]
    out = np.sum(0.5 * (y[1:] + y[:-1]) * dxs)
    return np.float32(out)



# revision 11
# speedup vs baseline: 1.7809x; 1.7809x over previous
"""CRPS loss kernel for Trainium2, 8 NeuronCores (SPMD data-parallel).

reference semantics:
    p, t = prediction.ravel(), target.ravel()       # N = 16,611,840 each
    lo, hi = min(min p, min t), max(max p, max t)
    x = linspace(lo, hi, 1000)  (f32)
    cdf_q(x_i) = #{v in q : v <= x_i} / N
    return trapz(|cdf_p - cdf_t|^2, x)

Device work (per core, 1/8 shard of each tensor):
  kernel A: running min/max reduce  -> per-core (min, -max)
  kernel B: per element j = rint(v*A + B) in [0, 1000]; digits
            m = j & 31, r = (j>>5) & 15, h = j >> 9.
            Joint (m, r) histogram via PACK4 block-diagonal PE matmuls:
            lhsT = plain one-hots of m (4 groups x 32 bins), moving = one-hots
            of r weighted by 4096^h (4 groups x 16 bins). PSUM accumulates 8
            chunks, then the two 4096-base fields are split into separate i32
            accumulators (exact: per-segment field counts < 4096).
Host: combine cores' [128, 256] i32 accs -> exact 1024-bin histogram,
      fold j>=999, cumsum, 1000-point trapz in f64.

Shards padded with the shard's first element to [128, 16384]; the host
subtracts the pad count from the padded value's bin (same f32 math).
"""

import numpy as np
from concourse import bacc, mybir, tile
from concourse.bass_utils import run_bass_kernel_spmd

P = 128
NCORES = 8
TOTAL = 16 * 1 * 721 * 1440          # 16,611,840
SHARD = TOTAL // NCORES              # 2,076,480
KTOT = 16384                         # padded columns/core/tensor
PADN = P * KTOT - SHARD              # 20,672
NX = 1000
C = 512                              # chunk columns
NCHUNK = KTOT // C                   # 32 per tensor
NI = C // 4                          # PACK4 matmuls per chunk
SEG = 8                              # chunks per psum accumulation segment
RED = 4096                           # minmax reduce chunk

F32 = mybir.dt.float32
I32 = mybir.dt.int32
BF16 = mybir.dt.bfloat16
ALU = mybir.AluOpType
ACT = mybir.ActivationFunctionType

# producer build split (of 32 plain is_equal + 16 weighted stt = 48)
N_POOL_PLAIN = 0      # Pool cannot run ALU ops on this ISA
N_ACT_PLAIN = 7       # plain builds done on Act (2-op square/relu)
POOL_TT = False       # Pool supports only copy/reduce/memset/DMA
ACT_ROUND = True      # affine+round via Act activation Identity (else DVE)


def _build_minmax():
    nc = bacc.Bacc()
    ins = [
        nc.declare_dram_parameter("pv", [P, KTOT], F32, isOutput=False),
        nc.declare_dram_parameter("tv", [P, KTOT], F32, isOutput=False),
    ]
    out = nc.declare_dram_parameter("mm", [1, 2], F32, isOutput=True)

    with tile.TileContext(nc) as tc:
        with (
            tc.tile_pool(name="sbuf", bufs=2) as pool,
            tc.tile_pool(name="acc", bufs=1) as apool,
        ):
            ntile = 2 * (KTOT // RED)            # 8 tiles
            NPOOL = 5                            # tiles whose MAX runs on Pool
            mins = apool.tile([P, ntile], F32)
            maxs = apool.tile([P, ntile - NPOOL], F32)
            pmax1 = apool.tile([1, NPOOL], F32)
            t = 0
            for src in ins:
                for ci in range(KTOT // RED):
                    v = pool.tile([P, RED], F32, tag="v")
                    nc.sync.dma_start(v[:], src[:, ci * RED:(ci + 1) * RED])
                    nc.vector.tensor_reduce(
                        mins[:, t:t + 1], v[:], mybir.AxisListType.X, ALU.min)
                    if t < NPOOL:
                        # whole-tile max on Pool -> [1,1] partial
                        nc.gpsimd.tensor_reduce(
                            pmax1[:, t:t + 1], v[:], mybir.AxisListType.XYZWC,
                            ALU.max)
                    else:
                        nc.vector.tensor_reduce(
                            maxs[:, t - NPOOL:t - NPOOL + 1], v[:],
                            mybir.AxisListType.X, ALU.max)
                    t += 1
            pmin = apool.tile([P, 1], F32)
            pmax = apool.tile([P, 1], F32)
            nc.vector.tensor_reduce(pmin[:], mins[:], mybir.AxisListType.X,
                                    ALU.min)
            nc.vector.tensor_reduce(pmax[:], maxs[:], mybir.AxisListType.X,
                                    ALU.max)
            both = apool.tile([P, 2], F32)
            nc.vector.tensor_scalar(out=both[:, 0:1], in0=pmin[:], scalar1=-1.0,
                                    scalar2=None, op0=ALU.mult)
            nc.vector.tensor_copy(out=both[:, 1:2], in_=pmax[:])
            red = apool.tile([1, 2], F32)
            nc.gpsimd.tensor_reduce(red[:], both[:], mybir.AxisListType.C,
                                    ALU.max)
            # fold in Pool max partials
            pb = apool.tile([1, 1], F32)
            nc.vector.tensor_reduce(pb[:], pmax1[:], mybir.AxisListType.X,
                                    ALU.max)
            fin = apool.tile([1, 2], F32)
            nc.vector.tensor_copy(out=fin[:], in_=red[:])
            nc.vector.tensor_tensor(out=fin[:, 1:2], in0=red[:, 1:2],
                                    in1=pb[:], op=ALU.max)
            nc.sync.dma_start(out[:], fin[:])
    nc.compile()
    return nc


def _build_hist():
    nc = bacc.Bacc()
    ins = [
        nc.declare_dram_parameter("pv", [P, KTOT], F32, isOutput=False),
        nc.declare_dram_parameter("tv", [P, KTOT], F32, isOutput=False),
    ]
    ab_in = nc.declare_dram_parameter("ab", [P, 2], F32, isOutput=False)
    # [0:64] acc0_p, [64:128] acc1_p, [128:192] acc0_t, [192:256] acc1_t
    out = nc.declare_dram_parameter("hist", [P, 256], I32, isOutput=True)

    with tile.TileContext(nc) as tc:
        with (
            tc.tile_pool(name="data", bufs=3) as dpool,
            tc.tile_pool(name="dig", bufs=2) as gpool,
            tc.tile_pool(name="oh", bufs=2) as ohpool,
            tc.tile_pool(name="const", bufs=1) as cpool,
            tc.tile_pool(name="acc", bufs=1) as apool,
            tc.tile_pool(name="psum", bufs=2, space="PSUM") as pp,
        ):
            ab_raw = cpool.tile([P, 2], F32)
            nc.sync.dma_start(ab_raw[:], ab_in[:])
            ab = cpool.tile([P, 2], F32)
            nc.vector.tensor_copy(out=ab[:], in_=ab_raw[:])
            ab_a = cpool.tile([P, 2], F32)
            nc.scalar.copy(out=ab_a[:], in_=ab_raw[:])
            # const APs for Act-engine builds: -q biases and -1.0 scale
            cneg = cpool.tile([P, N_ACT_PLAIN + 1], F32)
            for k in range(N_ACT_PLAIN):
                nc.vector.memset(cneg[:, k:k + 1], -float(N_POOL_PLAIN + k))
            nc.vector.memset(cneg[:, N_ACT_PLAIN:N_ACT_PLAIN + 1], -1.0)

            accs = apool.tile([P, 256], I32)
            nc.vector.memset(accs[:], 0)
            cbin = []
            for q in range(N_POOL_PLAIN if POOL_TT else 0):
                cb = cpool.tile([P, C], BF16, tag=f"cbin{q}")
                nc.vector.memset(cb[:], float(q))
                cbin.append(cb)

            nseg_chunks = []  # (tensor_idx, chunk_idx) stream
            for ti in range(2):
                for ci in range(NCHUNK):
                    nseg_chunks.append((ti, ci))

            ps_cur = None
            for si, (ti, ci) in enumerate(nseg_chunks):
                src = ins[ti]
                v = dpool.tile([P, C], F32, tag="v")
                nc.sync.dma_start(v[:], src[:, ci * C:(ci + 1) * C])

                ji = gpool.tile([P, C], I32, tag="ji")
                if ACT_ROUND:
                    nc.scalar.activation(out=ji[:], in_=v[:], func=ACT.Identity,
                                         scale=ab_a[:, 0:1], bias=ab_a[:, 1:2])
                else:
                    zf = gpool.tile([P, C], F32, tag="zf")
                    nc.vector.tensor_scalar(out=zf[:], in0=v[:],
                                            scalar1=ab[:, 0:1],
                                            scalar2=ab[:, 1:2],
                                            op0=ALU.mult, op1=ALU.add)
                    nc.vector.tensor_copy(out=ji[:], in_=zf[:])
                m32 = gpool.tile([P, C], I32, tag="m32")
                nc.vector.tensor_scalar(out=m32[:], in0=ji[:], scalar1=31,
                                        scalar2=None, op0=ALU.bitwise_and)
                rh32 = gpool.tile([P, C], I32, tag="rh32")
                nc.vector.tensor_scalar(out=rh32[:], in0=ji[:], scalar1=5,
                                        scalar2=None,
                                        op0=ALU.arith_shift_right)
                mb = gpool.tile([P, C], BF16, tag="mb")
                nc.gpsimd.tensor_copy(out=mb[:], in_=m32[:])
                rhb = gpool.tile([P, C], BF16, tag="rhb")
                nc.gpsimd.tensor_copy(out=rhb[:], in_=rh32[:])
                step = gpool.tile([P, C], BF16, tag="step")
                nc.vector.tensor_scalar(out=step[:], in0=rhb[:], scalar1=16.0,
                                        scalar2=None, op0=ALU.is_ge)
                wb = gpool.tile([P, C], BF16, tag="wb")
                nc.vector.tensor_scalar(out=wb[:], in0=step[:], scalar1=4095.0,
                                        scalar2=1.0, op0=ALU.mult, op1=ALU.add)
                rb = gpool.tile([P, C], BF16, tag="rb")
                nc.vector.scalar_tensor_tensor(out=rb[:], in0=step[:],
                                               scalar=-16.0, in1=rhb[:],
                                               op0=ALU.mult, op1=ALU.add)

                # one-hots, g4 layout: ohm[p, cc*128 + q*4 + g]
                ohm = ohpool.tile([P, 32 * C], BF16, tag="ohm")
                ohr = ohpool.tile([P, 16 * C], BF16, tag="ohr")
                ohm4 = ohm[:].rearrange("p (cc q g) -> p cc q g", q=32, g=4)
                ohr4 = ohr[:].rearrange("p (cc r g) -> p cc r g", r=16, g=4)
                scratch = gpool.tile([P, C], BF16, tag="scratch")
                for q in range(32):
                    dst = ohm4[:, :, q, :]
                    if q < N_POOL_PLAIN and POOL_TT:
                        nc.gpsimd.tensor_tensor(out=dst, in0=mb[:],
                                                in1=cbin[q][:],
                                                op=ALU.is_equal)
                    elif q < N_POOL_PLAIN + N_ACT_PLAIN:
                        k = q - N_POOL_PLAIN
                        nc.scalar.activation(out=scratch[:], in_=mb[:],
                                             func=ACT.Square, scale=1.0,
                                             bias=cneg[:, k:k + 1])
                        nc.scalar.activation(
                            out=dst, in_=scratch[:], func=ACT.Relu,
                            scale=cneg[:, N_ACT_PLAIN:N_ACT_PLAIN + 1],
                            bias=1.0)
                    else:
                        nc.vector.tensor_scalar(out=dst, in0=mb[:],
                                                scalar1=float(q), scalar2=None,
                                                op0=ALU.is_equal)
                for r in range(16):
                    dst = ohr4[:, :, r, :]
                    nc.vector.scalar_tensor_tensor(out=dst, in0=rb[:],
                                                   scalar=float(r), in1=wb[:],
                                                   op0=ALU.is_equal,
                                                   op1=ALU.mult)

                # PACK4 block-diag matmuls; psum accumulates SEG chunks.
                # lhsT/rhs are contiguous slices; column order is (q*4+g) /
                # (r*4+g) -- host unscrambles.
                if si % SEG == 0:
                    ps_cur = pp.tile([P, 64], F32, tag="ps")
                for cc in range(NI):
                    nc.tensor.matmul(
                        ps_cur[:],
                        lhsT=ohm[:, cc * 128:(cc + 1) * 128],
                        rhs=ohr[:, cc * 64:(cc + 1) * 64],
                        start=(si % SEG == 0 and cc == 0),
                        stop=(si % SEG == SEG - 1 and cc == NI - 1),
                    )

                if si % SEG == SEG - 1:
                    # field split: x = c0 + 4096*c1 -> i32, split, accumulate
                    off = 128 * ti
                    xi = gpool.tile([P, 64], I32, tag="xi")
                    nc.scalar.copy(out=xi[:], in_=ps_cur[:])
                    c1 = gpool.tile([P, 64], I32, tag="c1")
                    nc.vector.tensor_scalar(out=c1[:], in0=xi[:], scalar1=12,
                                            scalar2=None,
                                            op0=ALU.arith_shift_right)
                    c0 = gpool.tile([P, 64], I32, tag="c0")
                    nc.vector.tensor_scalar(out=c0[:], in0=xi[:], scalar1=4095,
                                            scalar2=None, op0=ALU.bitwise_and)
                    addeng = nc.gpsimd if POOL_TT else nc.vector
                    addeng.tensor_tensor(
                        out=accs[:, off:off + 64], in0=accs[:, off:off + 64],
                        in1=c0[:], op=ALU.add)
                    addeng.tensor_tensor(
                        out=accs[:, off + 64:off + 128],
                        in0=accs[:, off + 64:off + 128],
                        in1=c1[:], op=ALU.add)

            nc.sync.dma_start(out[:], accs[:])
    nc.compile()
    return nc


_KERNELS = {}


def _get_kernels():
    if "mm" not in _KERNELS:
        _KERNELS["mm"] = _build_minmax()
        _KERNELS["hist"] = _build_hist()
    return _KERNELS["mm"], _KERNELS["hist"]


def _shard(flat):
    """Split [TOTAL] -> per-core padded [P, KTOT] tiles + pad values."""
    tiles, pads = [], []
    for c in range(NCORES):
        s = flat[c * SHARD:(c + 1) * SHARD]
        v0 = s[0]
        t = np.concatenate([s, np.full(PADN, v0, s.dtype)]).reshape(P, KTOT)
        tiles.append(t)
        pads.append(v0)
    return tiles, pads


def _bin_of(v, A, B):
    """Replicate device binning for a scalar f32 value."""
    t1 = np.float32(np.float32(v) * A)
    z = np.float32(t1 + B)
    j = int(np.rint(np.float64(z)))
    return min(max(j, 0), 1024 - 1)


def _acc_to_hist(acc):
    """[P, 128] i32 (acc0 | acc1) -> [1024] f64 histogram.

    psum cell (m*4+g, r*4+g') holds group-g counts on the g==g' diagonal."""
    h = np.zeros(1024, np.float64)
    for fi in range(2):
        blk = acc[:, fi * 64:(fi + 1) * 64].astype(np.float64)
        X = blk.reshape(32, 4, 16, 4)                # [m, g, r, g']
        diag = X[:, np.arange(4), :, np.arange(4)]   # [g, m, r]
        cnt = diag.sum(axis=0)                       # [m, r]
        h[fi * 512:(fi + 1) * 512] = cnt.T.ravel()   # j = 512h + 32r + m
    return h


def kernel(prediction, target):
    nc_mm, nc_hist = _get_kernels()
    p = np.ascontiguousarray(np.asarray(prediction, dtype=np.float32).ravel())
    t = np.ascontiguousarray(np.asarray(target, dtype=np.float32).ravel())
    p_tiles, p_pads = _shard(p)
    t_tiles, t_pads = _shard(t)
    core_ids = list(range(NCORES))

    in_maps = [{"pv": p_tiles[c], "tv": t_tiles[c]} for c in core_ids]
    res = run_bass_kernel_spmd(nc_mm, in_maps, core_ids).results
    mm = np.stack([r["mm"][0] for r in res])        # [8, 2] = (-min, max)
    lo = np.float32(-(mm[:, 0].max()))
    hi = np.float32(mm[:, 1].max())

    dx = np.float32((hi - lo) / np.float32(NX - 1))
    A = np.float32(np.float32(1.0) / dx)
    B = np.float32(np.float32(-lo * A) + np.float32(0.5))
    ab = np.stack([np.full(P, A, np.float32), np.full(P, B, np.float32)],
                  axis=1)

    in_maps = [{"pv": p_tiles[c], "tv": t_tiles[c], "ab": ab}
               for c in core_ids]
    res = run_bass_kernel_spmd(nc_hist, in_maps, core_ids).results

    hp = np.zeros(1024, np.float64)
    ht = np.zeros(1024, np.float64)
    for c in core_ids:
        acc = res[c]["hist"]                        # [P, 256] i32
        hp += _acc_to_hist(acc[:, 0:128])
        ht += _acc_to_hist(acc[:, 128:256])
        hp[_bin_of(p_pads[c], A, B)] -= PADN
        ht[_bin_of(t_pads[c], A, B)] -= PADN

    hp[NX - 1] += hp[NX:].sum()
    ht[NX - 1] += ht[NX:].sum()
    cnt_p = np.cumsum(hp[:NX])
    cnt_t = np.cumsum(ht[:NX])

    n = np.float64(TOTAL)
    diff = np.abs(cnt_p / n - cnt_t / n)
    y = diff * diff
    x = np.linspace(np.float64(lo), np.float64(hi), NX)
    dxs = x[1:] - x[:-1]
    out = np.sum(0.5 * (y[1:] + y[:-1]) * dxs)
    return np.float32(out)


# revision 13
# speedup vs baseline: 1.8865x; 1.0593x over previous
"""CRPS loss kernel for Trainium2, 8 NeuronCores (SPMD data-parallel).

reference semantics:
    p, t = prediction.ravel(), target.ravel()       # N = 16,611,840 each
    lo, hi = min(min p, min t), max(max p, max t)
    x = linspace(lo, hi, 1000)  (f32)
    cdf_q(x_i) = #{v in q : v <= x_i} / N
    return trapz(|cdf_p - cdf_t|^2, x)

Device work (per core, 1/8 shard of each tensor):
  kernel A: min/max reduce (DVE X-reduces + Pool whole-tile max partials).
  kernel B: per element j = rint(v*A + B) in [0, 1000] (Act affine+round);
            digits m = j & 31 (DVE), rh = j >> 5 (Act scaled-round trick).
            Joint (m, rh) histogram via PACK4 block-diagonal PE matmuls:
            lhsT = one-hots of m (4 groups x 32 bins, column order m*4+g),
            rhs  = one-hots of rh (4 groups x 32 bins, order rh*4+g).
            PSUM accumulates the WHOLE tensor (counts < 2^24, exact f32);
            one psum->sbuf copy + DMA per tensor. The device's own binning
            of the pad value (partition 127 is all-pad) is exported as jpad
            so the host subtracts pads exactly.
Host: combine cores' [128, 256] f32 psum dumps -> exact 1024-bin histogram,
      fold j>=999, cumsum, 1000-point trapz in f64.
"""

import numpy as np
from concourse import bacc, mybir, tile
from concourse.bass_utils import run_bass_kernel_spmd

P = 128
NCORES = 8
TOTAL = 16 * 1 * 721 * 1440          # 16,611,840
SHARD = TOTAL // NCORES              # 2,076,480
KTOT = 16384                         # padded columns/core/tensor
PADN = P * KTOT - SHARD              # 20,672
NX = 1000
C = 512                              # chunk columns
NCHUNK = KTOT // C                   # 32 per tensor
NI = C // 4                          # PACK4 matmuls per chunk
RED = 4096                           # minmax reduce chunk

F32 = mybir.dt.float32
I32 = mybir.dt.int32
BF16 = mybir.dt.bfloat16
ALU = mybir.AluOpType
ACT = mybir.ActivationFunctionType

N_ACT_PLAIN = 10      # m-side bins built on Act (2-op square/relu)


def _build_minmax():
    nc = bacc.Bacc()
    ins = [
        nc.declare_dram_parameter("pv", [P, KTOT], F32, isOutput=False),
        nc.declare_dram_parameter("tv", [P, KTOT], F32, isOutput=False),
    ]
    out = nc.declare_dram_parameter("mm", [1, 2], F32, isOutput=True)

    with tile.TileContext(nc) as tc:
        with (
            tc.tile_pool(name="sbuf", bufs=4) as pool,
            tc.tile_pool(name="acc", bufs=1) as apool,
        ):
            ntile = 2 * (KTOT // RED)            # 8 tiles
            NPOOL = 5                            # tiles whose MAX runs on Pool
            mins = apool.tile([P, ntile], F32)
            maxs = apool.tile([P, ntile - NPOOL], F32)
            pmax1 = apool.tile([1, NPOOL], F32)
            t = 0
            for src in ins:
                for ci in range(KTOT // RED):
                    v = pool.tile([P, RED], F32, tag="v")
                    nc.sync.dma_start(v[:], src[:, ci * RED:(ci + 1) * RED])
                    nc.vector.tensor_reduce(
                        mins[:, t:t + 1], v[:], mybir.AxisListType.X, ALU.min)
                    if t < NPOOL:
                        nc.gpsimd.tensor_reduce(
                            pmax1[:, t:t + 1], v[:], mybir.AxisListType.XYZWC,
                            ALU.max)
                    else:
                        nc.vector.tensor_reduce(
                            maxs[:, t - NPOOL:t - NPOOL + 1], v[:],
                            mybir.AxisListType.X, ALU.max)
                    t += 1
            pmin = apool.tile([P, 1], F32)
            pmax = apool.tile([P, 1], F32)
            nc.vector.tensor_reduce(pmin[:], mins[:], mybir.AxisListType.X,
                                    ALU.min)
            nc.vector.tensor_reduce(pmax[:], maxs[:], mybir.AxisListType.X,
                                    ALU.max)
            both = apool.tile([P, 2], F32)
            nc.vector.tensor_scalar(out=both[:, 0:1], in0=pmin[:], scalar1=-1.0,
                                    scalar2=None, op0=ALU.mult)
            nc.vector.tensor_copy(out=both[:, 1:2], in_=pmax[:])
            red = apool.tile([1, 2], F32)
            nc.gpsimd.tensor_reduce(red[:], both[:], mybir.AxisListType.C,
                                    ALU.max)
            pb = apool.tile([1, 1], F32)
            nc.vector.tensor_reduce(pb[:], pmax1[:], mybir.AxisListType.X,
                                    ALU.max)
            fin = apool.tile([1, 2], F32)
            nc.vector.tensor_copy(out=fin[:], in_=red[:])
            nc.vector.tensor_tensor(out=fin[:, 1:2], in0=red[:, 1:2],
                                    in1=pb[:], op=ALU.max)
            nc.sync.dma_start(out[:], fin[:])
    nc.compile()
    return nc


def _build_hist():
    nc = bacc.Bacc()
    ins = [
        nc.declare_dram_parameter("pv", [P, KTOT], F32, isOutput=False),
        nc.declare_dram_parameter("tv", [P, KTOT], F32, isOutput=False),
    ]
    ab_in = nc.declare_dram_parameter("ab", [P, 2], F32, isOutput=False)
    # raw psum dumps: [0:128] prediction, [128:256] target
    out = nc.declare_dram_parameter("hist", [P, 256], F32, isOutput=True)
    out_jp = nc.declare_dram_parameter("jpad", [1, 2], I32, isOutput=True)

    with tile.TileContext(nc) as tc:
        with (
            tc.tile_pool(name="data", bufs=3) as dpool,
            tc.tile_pool(name="dig", bufs=2) as gpool,
            tc.tile_pool(name="oh", bufs=2) as ohpool,
            tc.tile_pool(name="const", bufs=1) as cpool,
            tc.tile_pool(name="psum", bufs=1, space="PSUM") as pp,
        ):
            ab_raw = cpool.tile([P, 2], F32)
            nc.sync.dma_start(ab_raw[:], ab_in[:])
            ab_a = cpool.tile([P, 2], F32)
            nc.scalar.copy(out=ab_a[:], in_=ab_raw[:])
            # consts: rh-extract scale/bias, Act-build -q biases, -1.0 scale
            c32 = cpool.tile([P, 2], F32)
            nc.vector.memset(c32[:, 0:1], 0.03125)
            nc.vector.memset(c32[:, 1:2], -0.484375)
            cneg = cpool.tile([P, N_ACT_PLAIN + 1], F32)
            for k in range(N_ACT_PLAIN):
                nc.vector.memset(cneg[:, k:k + 1], -float(k))
            nc.vector.memset(cneg[:, N_ACT_PLAIN:N_ACT_PLAIN + 1], -1.0)

            ps_p = pp.tile([P, 128], F32, tag="psP")
            ps_t = pp.tile([P, 128], F32, tag="psT")
            ps = [ps_p, ps_t]

            chunks = [(ti, ci) for ti in range(2) for ci in range(NCHUNK)]

            def phase_a(si):
                ti, ci = chunks[si]
                v = dpool.tile([P, C], F32, tag="v")
                nc.sync.dma_start(v[:], ins[ti][:, ci * C:(ci + 1) * C])
                ji = gpool.tile([P, C], I32, tag="ji")
                nc.scalar.activation(out=ji[:], in_=v[:], func=ACT.Identity,
                                     scale=ab_a[:, 0:1], bias=ab_a[:, 1:2])
                if ci == 0:
                    # partition 127 is all padding: export device pad bin
                    nc.sync.dma_start(out_jp[:, ti:ti + 1],
                                      ji[127:128, 0:1])
                rh32 = gpool.tile([P, C], I32, tag="rh32")
                nc.scalar.activation(out=rh32[:], in_=ji[:], func=ACT.Identity,
                                     scale=c32[:, 0:1], bias=c32[:, 1:2])
                m32 = gpool.tile([P, C], I32, tag="m32")
                nc.vector.scalar_tensor_tensor(out=m32[:], in0=rh32[:],
                                               scalar=-32, in1=ji[:],
                                               op0=ALU.mult, op1=ALU.add)
                mb = gpool.tile([P, C], BF16, tag="mb")
                nc.gpsimd.tensor_copy(out=mb[:], in_=m32[:])
                rhb = gpool.tile([P, C], BF16, tag="rhb")
                nc.gpsimd.tensor_copy(out=rhb[:], in_=rh32[:])
                return mb, rhb

            def phase_b(si, mb, rhb):
                ti, ci = chunks[si]
                ohm = ohpool.tile([P, 32 * C], BF16, tag="ohm")
                ohr = ohpool.tile([P, 32 * C], BF16, tag="ohr")
                ohm4 = ohm[:].rearrange("p (cc q g) -> p cc q g", q=32, g=4)
                ohr4 = ohr[:].rearrange("p (cc q g) -> p cc q g", q=32, g=4)
                scratch = gpool.tile([P, C], BF16, tag="scratch")
                for q in range(32):
                    if q < N_ACT_PLAIN:
                        nc.scalar.activation(out=scratch[:], in_=mb[:],
                                             func=ACT.Square, scale=1.0,
                                             bias=cneg[:, q:q + 1])
                        nc.scalar.activation(
                            out=ohm4[:, :, q, :], in_=scratch[:],
                            func=ACT.Relu,
                            scale=cneg[:, N_ACT_PLAIN:N_ACT_PLAIN + 1],
                            bias=1.0)
                    else:
                        nc.vector.tensor_scalar(out=ohm4[:, :, q, :],
                                                in0=mb[:], scalar1=float(q),
                                                scalar2=None, op0=ALU.is_equal)
                for q in range(32):
                    nc.vector.tensor_scalar(out=ohr4[:, :, q, :], in0=rhb[:],
                                            scalar1=float(q), scalar2=None,
                                            op0=ALU.is_equal)
                for cc in range(NI):
                    nc.tensor.matmul(
                        ps[ti][:],
                        lhsT=ohm[:, cc * 128:(cc + 1) * 128],
                        rhs=ohr[:, cc * 128:(cc + 1) * 128],
                        start=(ci == 0 and cc == 0),
                        stop=(ci == NCHUNK - 1 and cc == NI - 1),
                    )
                if ci == NCHUNK - 1:
                    hsb = dpool.tile([P, 128], F32, tag="hsb")
                    nc.vector.tensor_copy(out=hsb[:], in_=ps[ti][:])
                    nc.sync.dma_start(out[:, ti * 128:(ti + 1) * 128], hsb[:])

            # software pipeline: A(si+1) emitted before B(si)
            cur = phase_a(0)
            for si in range(len(chunks)):
                nxt = phase_a(si + 1) if si + 1 < len(chunks) else None
                phase_b(si, *cur)
                cur = nxt
    nc.compile()
    return nc


_KERNELS = {}


def _get_kernels():
    if "mm" not in _KERNELS:
        _KERNELS["mm"] = _build_minmax()
        _KERNELS["hist"] = _build_hist()
    return _KERNELS["mm"], _KERNELS["hist"]


def _shard(flat):
    """Split [TOTAL] -> per-core padded [P, KTOT] tiles + pad values."""
    tiles, pads = [], []
    for c in range(NCORES):
        s = flat[c * SHARD:(c + 1) * SHARD]
        v0 = s[0]
        t = np.concatenate([s, np.full(PADN, v0, s.dtype)]).reshape(P, KTOT)
        tiles.append(t)
        pads.append(v0)
    return tiles, pads


def _psum_to_hist(X):
    """[P, 128] f32 psum dump -> [1024] f64 histogram.

    psum cell (m*4+g, rh*4+g') holds group-g counts on the g==g' diagonal."""
    Y = X.astype(np.float64).reshape(32, 4, 32, 4)   # [m, g, rh, g']
    diag = Y[:, np.arange(4), :, np.arange(4)]       # [g, m, rh]
    cnt = diag.sum(axis=0)                           # [m, rh]
    return cnt.T.ravel()                             # j = 32*rh + m


def kernel(prediction, target):
    nc_mm, nc_hist = _get_kernels()
    p = np.ascontiguousarray(np.asarray(prediction, dtype=np.float32).ravel())
    t = np.ascontiguousarray(np.asarray(target, dtype=np.float32).ravel())
    p_tiles, p_pads = _shard(p)
    t_tiles, t_pads = _shard(t)
    core_ids = list(range(NCORES))

    in_maps = [{"pv": p_tiles[c], "tv": t_tiles[c]} for c in core_ids]
    res = run_bass_kernel_spmd(nc_mm, in_maps, core_ids).results
    mm = np.stack([r["mm"][0] for r in res])        # [8, 2] = (-min, max)
    lo = np.float32(-(mm[:, 0].max()))
    hi = np.float32(mm[:, 1].max())

    dx = np.float32((hi - lo) / np.float32(NX - 1))
    A = np.float32(np.float32(1.0) / dx)
    B = np.float32(np.float32(-lo * A) + np.float32(0.5))
    ab = np.stack([np.full(P, A, np.float32), np.full(P, B, np.float32)],
                  axis=1)

    in_maps = [{"pv": p_tiles[c], "tv": t_tiles[c], "ab": ab}
               for c in core_ids]
    res = run_bass_kernel_spmd(nc_hist, in_maps, core_ids).results

    hp = np.zeros(1024, np.float64)
    ht = np.zeros(1024, np.float64)
    for c in core_ids:
        X = res[c]["hist"]                          # [P, 256] f32
        hp += _psum_to_hist(X[:, 0:128])
        ht += _psum_to_hist(X[:, 128:256])
        jp = res[c]["jpad"][0]                      # [2] i32, device pad bins
        hp[min(max(int(jp[0]), 0), 1023)] -= PADN
        ht[min(max(int(jp[1]), 0), 1023)] -= PADN

    hp[NX - 1] += hp[NX:].sum()
    ht[NX - 1] += ht[NX:].sum()
    cnt_p = np.cumsum(hp[:NX])
    cnt_t = np.cumsum(ht[:NX])

    n = np.float64(TOTAL)
    diff = np.abs(cnt_p / n - cnt_t / n)
    y = diff * diff
    x = np.linspace(np.float64(lo), np.float64(hi), NX)
    dxs = x[1:] - x[:-1]
    out = np.sum(0.5 * (y[1:] + y[:-1]) * dxs)
    return np.float32(out)


# revision 14
# speedup vs baseline: 2.1891x; 1.1604x over previous
"""CRPS loss kernel for Trainium2, 8 NeuronCores (SPMD data-parallel).

reference semantics:
    p, t = prediction.ravel(), target.ravel()       # N = 16,611,840 each
    lo, hi = min(min p, min t), max(max p, max t)
    x = linspace(lo, hi, 1000)  (f32)
    cdf_q(x_i) = #{v in q : v <= x_i} / N
    return trapz(|cdf_p - cdf_t|^2, x)

Device work (per core, 1/8 shard of each tensor):
  kernel A: min/max reduce (DVE X-reduces + Pool whole-tile max partials).
  kernel B: per element j = rint(v*A + B) in [0, 1000] (Act affine+round);
            digits m = j & 31 (DVE), rh = j >> 5 (Act scaled-round trick).
            Joint (m, rh) histogram via PACK4 block-diagonal PE matmuls:
            lhsT = one-hots of m (4 groups x 32 bins, column order m*4+g),
            rhs  = one-hots of rh (4 groups x 32 bins, order rh*4+g).
            PSUM accumulates the WHOLE tensor (counts < 2^24, exact f32);
            one psum->sbuf copy + DMA per tensor. The device's own binning
            of the pad value (partition 127 is all-pad) is exported as jpad
            so the host subtracts pads exactly.
Host: combine cores' [128, 256] f32 psum dumps -> exact 1024-bin histogram,
      fold j>=999, cumsum, 1000-point trapz in f64.
"""

import numpy as np
from concourse import bacc, mybir, tile
from concourse.bass_utils import run_bass_kernel_spmd

P = 128
NCORES = 8
TOTAL = 16 * 1 * 721 * 1440          # 16,611,840
SHARD = TOTAL // NCORES              # 2,076,480
KTOT = 16384                         # padded columns/core/tensor
PADN = P * KTOT - SHARD              # 20,672
NX = 1000
C = 512                              # chunk columns
NCHUNK = KTOT // C                   # 32 per tensor
NI = C // 4                          # PACK4 matmuls per chunk
RED = 4096                           # minmax reduce chunk

F32 = mybir.dt.float32
I32 = mybir.dt.int32
BF16 = mybir.dt.bfloat16
ALU = mybir.AluOpType
ACT = mybir.ActivationFunctionType

N_ACT_PLAIN = 8      # m-side bins built on Act (2-op square/relu)


def _build_minmax():
    nc = bacc.Bacc()
    ins = [
        nc.declare_dram_parameter("pv", [P, KTOT], F32, isOutput=False),
        nc.declare_dram_parameter("tv", [P, KTOT], F32, isOutput=False),
    ]
    out = nc.declare_dram_parameter("mm", [1, 2], F32, isOutput=True)

    with tile.TileContext(nc) as tc:
        with (
            tc.tile_pool(name="sbuf", bufs=4) as pool,
            tc.tile_pool(name="acc", bufs=1) as apool,
        ):
            ntile = 2 * (KTOT // RED)            # 8 tiles
            NPOOL = 5                            # tiles whose MAX runs on Pool
            mins = apool.tile([P, ntile], F32)
            maxs = apool.tile([P, ntile - NPOOL], F32)
            pmax1 = apool.tile([1, NPOOL], F32)
            t = 0
            for src in ins:
                for ci in range(KTOT // RED):
                    v = pool.tile([P, RED], F32, tag="v")
                    nc.sync.dma_start(v[:], src[:, ci * RED:(ci + 1) * RED])
                    nc.vector.tensor_reduce(
                        mins[:, t:t + 1], v[:], mybir.AxisListType.X, ALU.min)
                    if t < NPOOL:
                        nc.gpsimd.tensor_reduce(
                            pmax1[:, t:t + 1], v[:], mybir.AxisListType.XYZWC,
                            ALU.max)
                    else:
                        nc.vector.tensor_reduce(
                            maxs[:, t - NPOOL:t - NPOOL + 1], v[:],
                            mybir.AxisListType.X, ALU.max)
                    t += 1
            pmin = apool.tile([P, 1], F32)
            pmax = apool.tile([P, 1], F32)
            nc.vector.tensor_reduce(pmin[:], mins[:], mybir.AxisListType.X,
                                    ALU.min)
            nc.vector.tensor_reduce(pmax[:], maxs[:], mybir.AxisListType.X,
                                    ALU.max)
            both = apool.tile([P, 2], F32)
            nc.vector.tensor_scalar(out=both[:, 0:1], in0=pmin[:], scalar1=-1.0,
                                    scalar2=None, op0=ALU.mult)
            nc.vector.tensor_copy(out=both[:, 1:2], in_=pmax[:])
            red = apool.tile([1, 2], F32)
            nc.gpsimd.tensor_reduce(red[:], both[:], mybir.AxisListType.C,
                                    ALU.max)
            pb = apool.tile([1, 1], F32)
            nc.vector.tensor_reduce(pb[:], pmax1[:], mybir.AxisListType.X,
                                    ALU.max)
            fin = apool.tile([1, 2], F32)
            nc.vector.tensor_copy(out=fin[:], in_=red[:])
            nc.vector.tensor_tensor(out=fin[:, 1:2], in0=red[:, 1:2],
                                    in1=pb[:], op=ALU.max)
            nc.sync.dma_start(out[:], fin[:])
    nc.compile()
    return nc


def _build_hist():
    nc = bacc.Bacc()
    ins = [
        nc.declare_dram_parameter("pv", [P, KTOT], F32, isOutput=False),
        nc.declare_dram_parameter("tv", [P, KTOT], F32, isOutput=False),
    ]
    ab_in = nc.declare_dram_parameter("ab", [P, 2], F32, isOutput=False)
    # raw psum dumps: [0:128] prediction, [128:256] target
    out = nc.declare_dram_parameter("hist", [P, 256], F32, isOutput=True)
    out_jp = nc.declare_dram_parameter("jpad", [1, 2], I32, isOutput=True)

    with tile.TileContext(nc) as tc:
        with (
            tc.tile_pool(name="data", bufs=3) as dpool,
            tc.tile_pool(name="dig", bufs=2) as gpool,
            tc.tile_pool(name="oh", bufs=2) as ohpool,
            tc.tile_pool(name="const", bufs=1) as cpool,
            tc.tile_pool(name="psum", bufs=1, space="PSUM") as pp,
        ):
            ab_raw = cpool.tile([P, 2], F32)
            nc.sync.dma_start(ab_raw[:], ab_in[:])
            ab_a = cpool.tile([P, 2], F32)
            nc.scalar.copy(out=ab_a[:], in_=ab_raw[:])
            # consts: rh-extract scale/bias, Act-build -q biases, -1.0 scale
            c32 = cpool.tile([P, 2], F32)
            nc.vector.memset(c32[:, 0:1], 0.03125)
            nc.vector.memset(c32[:, 1:2], -0.484375)
            cneg = cpool.tile([P, N_ACT_PLAIN + 1], F32)
            for k in range(N_ACT_PLAIN):
                nc.vector.memset(cneg[:, k:k + 1], -float(k))
            nc.vector.memset(cneg[:, N_ACT_PLAIN:N_ACT_PLAIN + 1], -1.0)

            ps_p = pp.tile([P, 128], F32, tag="psP")
            ps_t = pp.tile([P, 128], F32, tag="psT")
            ps = [ps_p, ps_t]

            chunks = [(ti, ci) for ti in range(2) for ci in range(NCHUNK)]

            def phase_a(si):
                ti, ci = chunks[si]
                v = dpool.tile([P, C], F32, tag="v")
                nc.sync.dma_start(v[:], ins[ti][:, ci * C:(ci + 1) * C])
                ji = gpool.tile([P, C], I32, tag="ji")
                nc.scalar.activation(out=ji[:], in_=v[:], func=ACT.Identity,
                                     scale=ab_a[:, 0:1], bias=ab_a[:, 1:2])
                if ci == 0:
                    # partition 127 is all padding: export device pad bin
                    nc.sync.dma_start(out_jp[:, ti:ti + 1],
                                      ji[127:128, 0:1])
                rh32 = gpool.tile([P, C], I32, tag="rh32")
                nc.scalar.activation(out=rh32[:], in_=ji[:], func=ACT.Identity,
                                     scale=c32[:, 0:1], bias=c32[:, 1:2])
                m32 = gpool.tile([P, C], I32, tag="m32")
                nc.vector.tensor_scalar(out=m32[:], in0=ji[:], scalar1=31,
                                        scalar2=None, op0=ALU.bitwise_and)
                mb = gpool.tile([P, C], BF16, tag="mb")
                nc.vector.tensor_copy(out=mb[:], in_=m32[:])
                rhb = gpool.tile([P, C], BF16, tag="rhb")
                nc.vector.tensor_copy(out=rhb[:], in_=rh32[:])
                return mb, rhb

            def phase_b(si, mb, rhb):
                ti, ci = chunks[si]
                ohm = ohpool.tile([P, 32 * C], BF16, tag="ohm")
                ohr = ohpool.tile([P, 32 * C], BF16, tag="ohr")
                ohm4 = ohm[:].rearrange("p (cc q g) -> p cc q g", q=32, g=4)
                ohr4 = ohr[:].rearrange("p (cc q g) -> p cc q g", q=32, g=4)
                scratch = gpool.tile([P, C], BF16, tag="scratch")
                for q in range(32):
                    if q < N_ACT_PLAIN:
                        nc.scalar.activation(out=scratch[:], in_=mb[:],
                                             func=ACT.Square, scale=1.0,
                                             bias=cneg[:, q:q + 1])
                        nc.scalar.activation(
                            out=ohm4[:, :, q, :], in_=scratch[:],
                            func=ACT.Relu,
                            scale=cneg[:, N_ACT_PLAIN:N_ACT_PLAIN + 1],
                            bias=1.0)
                    else:
                        nc.vector.tensor_scalar(out=ohm4[:, :, q, :],
                                                in0=mb[:], scalar1=float(q),
                                                scalar2=None, op0=ALU.is_equal)
                for q in range(32):
                    nc.vector.tensor_scalar(out=ohr4[:, :, q, :], in0=rhb[:],
                                            scalar1=float(q), scalar2=None,
                                            op0=ALU.is_equal)
                for cc in range(NI):
                    nc.tensor.matmul(
                        ps[ti][:],
                        lhsT=ohm[:, cc * 128:(cc + 1) * 128],
                        rhs=ohr[:, cc * 128:(cc + 1) * 128],
                        start=(ci == 0 and cc == 0),
                        stop=(ci == NCHUNK - 1 and cc == NI - 1),
                    )
                if ci == NCHUNK - 1:
                    hsb = dpool.tile([P, 128], F32, tag="hsb")
                    nc.vector.tensor_copy(out=hsb[:], in_=ps[ti][:])
                    nc.sync.dma_start(out[:, ti * 128:(ti + 1) * 128], hsb[:])

            # software pipeline: A(si+1) emitted before B(si)
            cur = phase_a(0)
            for si in range(len(chunks)):
                nxt = phase_a(si + 1) if si + 1 < len(chunks) else None
                phase_b(si, *cur)
                cur = nxt
    nc.compile()
    return nc


_KERNELS = {}


def _get_kernels():
    if "mm" not in _KERNELS:
        _KERNELS["mm"] = _build_minmax()
        _KERNELS["hist"] = _build_hist()
    return _KERNELS["mm"], _KERNELS["hist"]


def _shard(flat):
    """Split [TOTAL] -> per-core padded [P, KTOT] tiles + pad values."""
    tiles, pads = [], []
    for c in range(NCORES):
        s = flat[c * SHARD:(c + 1) * SHARD]
        v0 = s[0]
        t = np.concatenate([s, np.full(PADN, v0, s.dtype)]).reshape(P, KTOT)
        tiles.append(t)
        pads.append(v0)
    return tiles, pads


def _psum_to_hist(X):
    """[P, 128] f32 psum dump -> [1024] f64 histogram.

    psum cell (m*4+g, rh*4+g') holds group-g counts on the g==g' diagonal."""
    Y = X.astype(np.float64).reshape(32, 4, 32, 4)   # [m, g, rh, g']
    diag = Y[:, np.arange(4), :, np.arange(4)]       # [g, m, rh]
    cnt = diag.sum(axis=0)                           # [m, rh]
    return cnt.T.ravel()                             # j = 32*rh + m


def kernel(prediction, target):
    nc_mm, nc_hist = _get_kernels()
    p = np.ascontiguousarray(np.asarray(prediction, dtype=np.float32).ravel())
    t = np.ascontiguousarray(np.asarray(target, dtype=np.float32).ravel())
    p_tiles, p_pads = _shard(p)
    t_tiles, t_pads = _shard(t)
    core_ids = list(range(NCORES))

    in_maps = [{"pv": p_tiles[c], "tv": t_tiles[c]} for c in core_ids]
    res = run_bass_kernel_spmd(nc_mm, in_maps, core_ids).results
    mm = np.stack([r["mm"][0] for r in res])        # [8, 2] = (-min, max)
    lo = np.float32(-(mm[:, 0].max()))
    hi = np.float32(mm[:, 1].max())

    dx = np.float32((hi - lo) / np.float32(NX - 1))
    A = np.float32(np.float32(1.0) / dx)
    B = np.float32(np.float32(-lo * A) + np.float32(0.5))
    ab = np.stack([np.full(P, A, np.float32), np.full(P, B, np.float32)],
                  axis=1)

    in_maps = [{"pv": p_tiles[c], "tv": t_tiles[c], "ab": ab}
               for c in core_ids]
    res = run_bass_kernel_spmd(nc_hist, in_maps, core_ids).results

    hp = np.zeros(1024, np.float64)
    ht = np.zeros(1024, np.float64)
    for c in core_ids:
        X = res[c]["hist"]                          # [P, 256] f32
        hp += _psum_to_hist(X[:, 0:128])
        ht += _psum_to_hist(X[:, 128:256])
        jp = res[c]["jpad"][0]                      # [2] i32, device pad bins
        hp[min(max(int(jp[0]), 0), 1023)] -= PADN
        ht[min(max(int(jp[1]), 0), 1023)] -= PADN

    hp[NX - 1] += hp[NX:].sum()
    ht[NX - 1] += ht[NX:].sum()
    cnt_p = np.cumsum(hp[:NX])
    cnt_t = np.cumsum(ht[:NX])

    n = np.float64(TOTAL)
    diff = np.abs(cnt_p / n - cnt_t / n)
    y = diff * diff
    x = np.linspace(np.float64(lo), np.float64(hi), NX)
    dxs = x[1:] - x[:-1]
    out = np.sum(0.5 * (y[1:] + y[:-1]) * dxs)
    return np.float32(out)


# revision 15
# speedup vs baseline: 2.2427x; 1.0245x over previous
"""CRPS loss kernel for Trainium2, 8 NeuronCores (SPMD data-parallel).

reference semantics:
    p, t = prediction.ravel(), target.ravel()       # N = 16,611,840 each
    lo, hi = min(min p, min t), max(max p, max t)
    x = linspace(lo, hi, 1000)  (f32)
    cdf_q(x_i) = #{v in q : v <= x_i} / N
    return trapz(|cdf_p - cdf_t|^2, x)

Device work (per core, 1/8 shard of each tensor):
  kernel A: min/max reduce (DVE X-reduces + Pool whole-tile max partials).
  kernel B: per element j = rint(v*A + B) in [0, 1000] (Act affine+round);
            digits m = j & 31 (DVE), rh = j >> 5 (Act scaled-round trick).
            Joint (m, rh) histogram via PACK4 block-diagonal PE matmuls:
            lhsT = one-hots of m (4 groups x 32 bins, column order m*4+g),
            rhs  = one-hots of rh (4 groups x 32 bins, order rh*4+g).
            PSUM accumulates the WHOLE tensor (counts < 2^24, exact f32);
            one psum->sbuf copy + DMA per tensor. The device's own binning
            of the pad value (partition 127 is all-pad) is exported as jpad
            so the host subtracts pads exactly.
Host: combine cores' [128, 256] f32 psum dumps -> exact 1024-bin histogram,
      fold j>=999, cumsum, 1000-point trapz in f64.
"""

import numpy as np
from concourse import bacc, mybir, tile
from concourse.bass_utils import run_bass_kernel_spmd

P = 128
NCORES = 8
TOTAL = 16 * 1 * 721 * 1440          # 16,611,840
SHARD = TOTAL // NCORES              # 2,076,480
KTOT = 16384                         # padded columns/core/tensor
PADN = P * KTOT - SHARD              # 20,672
NX = 1000
C = 512                              # chunk columns
NCHUNK = KTOT // C                   # 32 per tensor
NI = C // 4                          # PACK4 matmuls per chunk
RED = 4096                           # minmax reduce chunk

F32 = mybir.dt.float32
I32 = mybir.dt.int32
BF16 = mybir.dt.bfloat16
ALU = mybir.AluOpType
ACT = mybir.ActivationFunctionType

N_ACT_PLAIN = 7      # m-side bins built on Act (2-op square/relu)


def _build_minmax():
    nc = bacc.Bacc()
    ins = [
        nc.declare_dram_parameter("pv", [P, KTOT], F32, isOutput=False),
        nc.declare_dram_parameter("tv", [P, KTOT], F32, isOutput=False),
    ]
    out = nc.declare_dram_parameter("mm", [1, 2], F32, isOutput=True)

    with tile.TileContext(nc) as tc:
        with (
            tc.tile_pool(name="sbuf", bufs=6) as pool,
            tc.tile_pool(name="acc", bufs=1) as apool,
        ):
            ntile = 2 * (KTOT // RED)            # 8 tiles
            NPOOL = 5                            # tiles whose MAX runs on Pool
            mins = apool.tile([P, ntile], F32)
            maxs = apool.tile([P, ntile - NPOOL], F32)
            pmax1 = apool.tile([1, NPOOL], F32)
            t = 0
            for src in ins:
                for ci in range(KTOT // RED):
                    v = pool.tile([P, RED], F32, tag="v")
                    dmaeng = nc.sync if t % 2 == 0 else nc.scalar
                    dmaeng.dma_start(v[:], src[:, ci * RED:(ci + 1) * RED])
                    nc.vector.tensor_reduce(
                        mins[:, t:t + 1], v[:], mybir.AxisListType.X, ALU.min)
                    if t < NPOOL:
                        nc.gpsimd.tensor_reduce(
                            pmax1[:, t:t + 1], v[:], mybir.AxisListType.XYZWC,
                            ALU.max)
                    else:
                        nc.vector.tensor_reduce(
                            maxs[:, t - NPOOL:t - NPOOL + 1], v[:],
                            mybir.AxisListType.X, ALU.max)
                    t += 1
            pmin = apool.tile([P, 1], F32)
            pmax = apool.tile([P, 1], F32)
            nc.vector.tensor_reduce(pmin[:], mins[:], mybir.AxisListType.X,
                                    ALU.min)
            nc.vector.tensor_reduce(pmax[:], maxs[:], mybir.AxisListType.X,
                                    ALU.max)
            both = apool.tile([P, 2], F32)
            nc.vector.tensor_scalar(out=both[:, 0:1], in0=pmin[:], scalar1=-1.0,
                                    scalar2=None, op0=ALU.mult)
            nc.vector.tensor_copy(out=both[:, 1:2], in_=pmax[:])
            red = apool.tile([1, 2], F32)
            nc.gpsimd.tensor_reduce(red[:], both[:], mybir.AxisListType.C,
                                    ALU.max)
            pb = apool.tile([1, 1], F32)
            nc.vector.tensor_reduce(pb[:], pmax1[:], mybir.AxisListType.X,
                                    ALU.max)
            fin = apool.tile([1, 2], F32)
            nc.vector.tensor_copy(out=fin[:], in_=red[:])
            nc.vector.tensor_tensor(out=fin[:, 1:2], in0=red[:, 1:2],
                                    in1=pb[:], op=ALU.max)
            nc.sync.dma_start(out[:], fin[:])
    nc.compile()
    return nc


def _build_hist():
    nc = bacc.Bacc()
    ins = [
        nc.declare_dram_parameter("pv", [P, KTOT], F32, isOutput=False),
        nc.declare_dram_parameter("tv", [P, KTOT], F32, isOutput=False),
    ]
    ab_in = nc.declare_dram_parameter("ab", [P, 2], F32, isOutput=False)
    # raw psum dumps: [0:128] prediction, [128:256] target
    out = nc.declare_dram_parameter("hist", [P, 256], F32, isOutput=True)
    out_jp = nc.declare_dram_parameter("jpad", [1, 2], I32, isOutput=True)

    with tile.TileContext(nc) as tc:
        with (
            tc.tile_pool(name="data", bufs=3) as dpool,
            tc.tile_pool(name="dig", bufs=2) as gpool,
            tc.tile_pool(name="oh", bufs=2) as ohpool,
            tc.tile_pool(name="const", bufs=1) as cpool,
            tc.tile_pool(name="psum", bufs=1, space="PSUM") as pp,
        ):
            ab_raw = cpool.tile([P, 2], F32)
            nc.sync.dma_start(ab_raw[:], ab_in[:])
            ab_a = cpool.tile([P, 2], F32)
            nc.scalar.copy(out=ab_a[:], in_=ab_raw[:])
            # consts: rh-extract scale/bias, Act-build -q biases, -1.0 scale
            c32 = cpool.tile([P, 2], F32)
            nc.vector.memset(c32[:, 0:1], 0.03125)
            nc.vector.memset(c32[:, 1:2], -0.484375)
            cneg = cpool.tile([P, N_ACT_PLAIN + 1], F32)
            for k in range(N_ACT_PLAIN):
                nc.vector.memset(cneg[:, k:k + 1], -float(k))
            nc.vector.memset(cneg[:, N_ACT_PLAIN:N_ACT_PLAIN + 1], -1.0)

            ps_p = pp.tile([P, 128], F32, tag="psP")
            ps_t = pp.tile([P, 128], F32, tag="psT")
            ps = [ps_p, ps_t]

            chunks = [(ti, ci) for ti in range(2) for ci in range(NCHUNK)]

            def phase_a(si):
                ti, ci = chunks[si]
                v = dpool.tile([P, C], F32, tag="v")
                nc.sync.dma_start(v[:], ins[ti][:, ci * C:(ci + 1) * C])
                ji = gpool.tile([P, C], I32, tag="ji")
                nc.scalar.activation(out=ji[:], in_=v[:], func=ACT.Identity,
                                     scale=ab_a[:, 0:1], bias=ab_a[:, 1:2])
                if ci == 0:
                    # partition 127 is all padding: export device pad bin
                    nc.sync.dma_start(out_jp[:, ti:ti + 1],
                                      ji[127:128, 0:1])
                rh32 = gpool.tile([P, C], I32, tag="rh32")
                nc.scalar.activation(out=rh32[:], in_=ji[:], func=ACT.Identity,
                                     scale=c32[:, 0:1], bias=c32[:, 1:2])
                m32 = gpool.tile([P, C], I32, tag="m32")
                nc.vector.tensor_scalar(out=m32[:], in0=ji[:], scalar1=31,
                                        scalar2=None, op0=ALU.bitwise_and)
                mb = gpool.tile([P, C], BF16, tag="mb")
                nc.gpsimd.tensor_copy(out=mb[:], in_=m32[:])
                rhb = gpool.tile([P, C], BF16, tag="rhb")
                nc.gpsimd.tensor_copy(out=rhb[:], in_=rh32[:])
                return mb, rhb

            def phase_b(si, mb, rhb):
                ti, ci = chunks[si]
                ohm = ohpool.tile([P, 32 * C], BF16, tag="ohm")
                ohr = ohpool.tile([P, 32 * C], BF16, tag="ohr")
                ohm4 = ohm[:].rearrange("p (cc q g) -> p cc q g", q=32, g=4)
                ohr4 = ohr[:].rearrange("p (cc q g) -> p cc q g", q=32, g=4)
                scratch = gpool.tile([P, C], BF16, tag="scratch")
                for q in range(32):
                    if q < N_ACT_PLAIN:
                        nc.scalar.activation(out=scratch[:], in_=mb[:],
                                             func=ACT.Square, scale=1.0,
                                             bias=cneg[:, q:q + 1])
                        nc.scalar.activation(
                            out=ohm4[:, :, q, :], in_=scratch[:],
                            func=ACT.Relu,
                            scale=cneg[:, N_ACT_PLAIN:N_ACT_PLAIN + 1],
                            bias=1.0)
                    else:
                        nc.vector.tensor_scalar(out=ohm4[:, :, q, :],
                                                in0=mb[:], scalar1=float(q),
                                                scalar2=None, op0=ALU.is_equal)
                for q in range(32):
                    nc.vector.tensor_scalar(out=ohr4[:, :, q, :], in0=rhb[:],
                                            scalar1=float(q), scalar2=None,
                                            op0=ALU.is_equal)
                for cc in range(NI):
                    nc.tensor.matmul(
                        ps[ti][:],
                        lhsT=ohm[:, cc * 128:(cc + 1) * 128],
                        rhs=ohr[:, cc * 128:(cc + 1) * 128],
                        start=(ci == 0 and cc == 0),
                        stop=(ci == NCHUNK - 1 and cc == NI - 1),
                    )
                if ci == NCHUNK - 1:
                    hsb = dpool.tile([P, 128], F32, tag="hsb")
                    nc.vector.tensor_copy(out=hsb[:], in_=ps[ti][:])
                    nc.sync.dma_start(out[:, ti * 128:(ti + 1) * 128], hsb[:])

            # software pipeline: A(si+1) emitted before B(si)
            cur = phase_a(0)
            for si in range(len(chunks)):
                nxt = phase_a(si + 1) if si + 1 < len(chunks) else None
                phase_b(si, *cur)
                cur = nxt
    nc.compile()
    return nc


_KERNELS = {}


def _get_kernels():
    if "mm" not in _KERNELS:
        _KERNELS["mm"] = _build_minmax()
        _KERNELS["hist"] = _build_hist()
    return _KERNELS["mm"], _KERNELS["hist"]


def _shard(flat):
    """Split [TOTAL] -> per-core padded [P, KTOT] tiles + pad values."""
    tiles, pads = [], []
    for c in range(NCORES):
        s = flat[c * SHARD:(c + 1) * SHARD]
        v0 = s[0]
        t = np.concatenate([s, np.full(PADN, v0, s.dtype)]).reshape(P, KTOT)
        tiles.append(t)
        pads.append(v0)
    return tiles, pads


def _psum_to_hist(X):
    """[P, 128] f32 psum dump -> [1024] f64 histogram.

    psum cell (m*4+g, rh*4+g') holds group-g counts on the g==g' diagonal."""
    Y = X.astype(np.float64).reshape(32, 4, 32, 4)   # [m, g, rh, g']
    diag = Y[:, np.arange(4), :, np.arange(4)]       # [g, m, rh]
    cnt = diag.sum(axis=0)                           # [m, rh]
    return cnt.T.ravel()                             # j = 32*rh + m


def kernel(prediction, target):
    nc_mm, nc_hist = _get_kernels()
    p = np.ascontiguousarray(np.asarray(prediction, dtype=np.float32).ravel())
    t = np.ascontiguousarray(np.asarray(target, dtype=np.float32).ravel())
    p_tiles, p_pads = _shard(p)
    t_tiles, t_pads = _shard(t)
    core_ids = list(range(NCORES))

    in_maps = [{"pv": p_tiles[c], "tv": t_tiles[c]} for c in core_ids]
    res = run_bass_kernel_spmd(nc_mm, in_maps, core_ids).results
    mm = np.stack([r["mm"][0] for r in res])        # [8, 2] = (-min, max)
    lo = np.float32(-(mm[:, 0].max()))
    hi = np.float32(mm[:, 1].max())

    dx = np.float32((hi - lo) / np.float32(NX - 1))
    A = np.float32(np.float32(1.0) / dx)
    B = np.float32(np.float32(-lo * A) + np.float32(0.5))
    ab = np.stack([np.full(P, A, np.float32), np.full(P, B, np.float32)],
                  axis=1)

    in_maps = [{"pv": p_tiles[c], "tv": t_tiles[c], "ab": ab}
               for c in core_ids]
    res = run_bass_kernel_spmd(nc_hist, in_maps, core_ids).results

    hp = np.zeros(1024, np.float64)
    ht = np.zeros(1024, np.float64)
    for c in core_ids:
        X = res[c]["hist"]                          # [P, 256] f32
        hp += _psum_to_hist(X[:, 0:128])
        ht += _psum_to_hist(X[:, 128:256])
        jp = res[c]["jpad"][0]                      # [2] i32, device pad bins
        hp[min(max(int(jp[0]), 0), 1023)] -= PADN
        ht[min(max(int(jp[1]), 0), 1023)] -= PADN

    hp[NX - 1] += hp[NX:].sum()
    ht[NX - 1] += ht[NX:].sum()
    cnt_p = np.cumsum(hp[:NX])
    cnt_t = np.cumsum(ht[:NX])

    n = np.float64(TOTAL)
    diff = np.abs(cnt_p / n - cnt_t / n)
    y = diff * diff
    x = np.linspace(np.float64(lo), np.float64(hi), NX)
    dxs = x[1:] - x[:-1]
    out = np.sum(0.5 * (y[1:] + y[:-1]) * dxs)
    return np.float32(out)


# revision 16
# speedup vs baseline: 2.4749x; 1.1036x over previous
"""CRPS loss kernel for Trainium2, 8 NeuronCores (SPMD data-parallel).

reference semantics:
    p, t = prediction.ravel(), target.ravel()       # N = 16,611,840 each
    lo, hi = min(min p, min t), max(max p, max t)
    x = linspace(lo, hi, 1000)  (f32)
    cdf_q(x_i) = #{v in q : v <= x_i} / N
    return trapz(|cdf_p - cdf_t|^2, x)

Device work (per core, 1/8 shard of each tensor):
  kernel A: min/max reduce (DVE X-reduces + Pool whole-tile max partials).
  kernel B: per element j = rint(v*A + B) in [0, 1000] (Act affine+round);
            digits m = j & 31 (DVE), rh = j >> 5 (Act scaled-round trick).
            Joint (m, rh) histogram via PACK4 block-diagonal PE matmuls:
            lhsT = one-hots of m (4 groups x 32 bins, column order m*4+g),
            rhs  = one-hots of rh (4 groups x 32 bins, order rh*4+g).
            PSUM accumulates the WHOLE tensor (counts < 2^24, exact f32);
            one psum->sbuf copy + DMA per tensor. The device's own binning
            of the pad value (partition 127 is all-pad) is exported as jpad
            so the host subtracts pads exactly.
Host: combine cores' [128, 256] f32 psum dumps -> exact 1024-bin histogram,
      fold j>=999, cumsum, 1000-point trapz in f64.
"""

import numpy as np
from concourse import bacc, mybir, tile
from concourse.bass_utils import run_bass_kernel_spmd

P = 128
NCORES = 8
TOTAL = 16 * 1 * 721 * 1440          # 16,611,840
SHARD = TOTAL // NCORES              # 2,076,480
KTOT = 16384                         # padded columns/core/tensor
PADN = P * KTOT - SHARD              # 20,672
NX = 1000
C = 512                              # chunk columns
NCHUNK = KTOT // C                   # 32 per tensor
NI = C // 4                          # PACK4 matmuls per chunk
RED = 4096                           # minmax reduce chunk

F32 = mybir.dt.float32
I32 = mybir.dt.int32
BF16 = mybir.dt.bfloat16
ALU = mybir.AluOpType
ACT = mybir.ActivationFunctionType

N_ACT_PLAIN = 7      # m-side bins built on Act (2-op square/relu)


def _build_minmax():
    nc = bacc.Bacc()
    ins = [
        nc.declare_dram_parameter("pv", [P, KTOT], F32, isOutput=False),
        nc.declare_dram_parameter("tv", [P, KTOT], F32, isOutput=False),
    ]
    out = nc.declare_dram_parameter("mm", [1, 2], F32, isOutput=True)

    with tile.TileContext(nc) as tc:
        with (
            tc.tile_pool(name="sbuf", bufs=6) as pool,
            tc.tile_pool(name="acc", bufs=1) as apool,
        ):
            ntile = 2 * (KTOT // RED)            # 8 tiles
            NPOOL = 5                            # tiles whose MAX runs on Pool
            mins = apool.tile([P, ntile], F32)
            maxs = apool.tile([P, ntile - NPOOL], F32)
            pmax1 = apool.tile([1, NPOOL], F32)
            t = 0
            for src in ins:
                for ci in range(KTOT // RED):
                    v = pool.tile([P, RED], F32, tag="v")
                    dmaeng = nc.sync if t % 2 == 0 else nc.scalar
                    dmaeng.dma_start(v[:], src[:, ci * RED:(ci + 1) * RED])
                    nc.vector.tensor_reduce(
                        mins[:, t:t + 1], v[:], mybir.AxisListType.X, ALU.min)
                    if t < NPOOL:
                        nc.gpsimd.tensor_reduce(
                            pmax1[:, t:t + 1], v[:], mybir.AxisListType.XYZWC,
                            ALU.max)
                    else:
                        nc.vector.tensor_reduce(
                            maxs[:, t - NPOOL:t - NPOOL + 1], v[:],
                            mybir.AxisListType.X, ALU.max)
                    t += 1
            pmin = apool.tile([P, 1], F32)
            pmax = apool.tile([P, 1], F32)
            nc.vector.tensor_reduce(pmin[:], mins[:], mybir.AxisListType.X,
                                    ALU.min)
            nc.vector.tensor_reduce(pmax[:], maxs[:], mybir.AxisListType.X,
                                    ALU.max)
            both = apool.tile([P, 2], F32)
            nc.vector.tensor_scalar(out=both[:, 0:1], in0=pmin[:], scalar1=-1.0,
                                    scalar2=None, op0=ALU.mult)
            nc.vector.tensor_copy(out=both[:, 1:2], in_=pmax[:])
            red = apool.tile([1, 2], F32)
            nc.gpsimd.tensor_reduce(red[:], both[:], mybir.AxisListType.C,
                                    ALU.max)
            pb = apool.tile([1, 1], F32)
            nc.vector.tensor_reduce(pb[:], pmax1[:], mybir.AxisListType.X,
                                    ALU.max)
            fin = apool.tile([1, 2], F32)
            nc.vector.tensor_copy(out=fin[:], in_=red[:])
            nc.vector.tensor_tensor(out=fin[:, 1:2], in0=red[:, 1:2],
                                    in1=pb[:], op=ALU.max)
            nc.sync.dma_start(out[:], fin[:])
    nc.compile()
    return nc


def _build_hist():
    nc = bacc.Bacc()
    ins = [
        nc.declare_dram_parameter("pv", [P, KTOT], F32, isOutput=False),
        nc.declare_dram_parameter("tv", [P, KTOT], F32, isOutput=False),
    ]
    ab_in = nc.declare_dram_parameter("ab", [P, 2], F32, isOutput=False)
    # raw psum dumps: [0:128] prediction, [128:256] target
    out = nc.declare_dram_parameter("hist", [P, 256], F32, isOutput=True)
    out_jp = nc.declare_dram_parameter("jpad", [1, 2], I32, isOutput=True)

    with tile.TileContext(nc) as tc:
        with (
            tc.tile_pool(name="data", bufs=3) as dpool,
            tc.tile_pool(name="dig", bufs=2) as gpool,
            tc.tile_pool(name="oh", bufs=2) as ohpool,
            tc.tile_pool(name="const", bufs=1) as cpool,
            tc.tile_pool(name="psum", bufs=1, space="PSUM") as pp,
        ):
            ab_raw = cpool.tile([P, 2], F32)
            nc.sync.dma_start(ab_raw[:], ab_in[:])
            ab_a = cpool.tile([P, 2], F32)
            nc.scalar.copy(out=ab_a[:], in_=ab_raw[:])
            # consts: rh-extract scale/bias, Act-build -q biases, -1.0 scale
            c32 = cpool.tile([P, 2], F32)
            nc.vector.memset(c32[:, 0:1], 0.03125)
            nc.vector.memset(c32[:, 1:2], -0.484375)
            cneg = cpool.tile([P, N_ACT_PLAIN + 1], F32)
            for k in range(N_ACT_PLAIN):
                nc.vector.memset(cneg[:, k:k + 1], -float(k))
            nc.vector.memset(cneg[:, N_ACT_PLAIN:N_ACT_PLAIN + 1], -1.0)

            ps_p = pp.tile([P, 128], F32, tag="psP")
            ps_t = pp.tile([P, 128], F32, tag="psT")
            ps = [ps_p, ps_t]

            chunks = [(ti, ci) for ti in range(2) for ci in range(NCHUNK)]

            def phase_a(si):
                ti, ci = chunks[si]
                v = dpool.tile([P, C], F32, tag="v")
                nc.sync.dma_start(v[:], ins[ti][:, ci * C:(ci + 1) * C])
                ji = gpool.tile([P, C], I32, tag="ji")
                nc.scalar.activation(out=ji[:], in_=v[:], func=ACT.Identity,
                                     scale=ab_a[:, 0:1], bias=ab_a[:, 1:2])
                if ci == 0:
                    # partition 127 is all padding: export device pad bin
                    nc.sync.dma_start(out_jp[:, ti:ti + 1],
                                      ji[127:128, 0:1])
                rh32 = gpool.tile([P, C], I32, tag="rh32")
                nc.scalar.activation(out=rh32[:], in_=ji[:], func=ACT.Identity,
                                     scale=c32[:, 0:1], bias=c32[:, 1:2])
                m32 = gpool.tile([P, C], I32, tag="m32")
                nc.vector.tensor_scalar(out=m32[:], in0=ji[:], scalar1=31,
                                        scalar2=None, op0=ALU.bitwise_and)
                mb = gpool.tile([P, C], BF16, tag="mb")
                nc.vector.tensor_copy(out=mb[:], in_=m32[:])
                rhb = gpool.tile([P, C], BF16, tag="rhb")
                nc.gpsimd.tensor_copy(out=rhb[:], in_=rh32[:])
                return mb, rhb

            def phase_b(si, mb, rhb):
                ti, ci = chunks[si]
                ohm = ohpool.tile([P, 32 * C], BF16, tag="ohm")
                ohr = ohpool.tile([P, 32 * C], BF16, tag="ohr")
                ohm4 = ohm[:].rearrange("p (cc q g) -> p cc q g", q=32, g=4)
                ohr4 = ohr[:].rearrange("p (cc q g) -> p cc q g", q=32, g=4)
                scratch = gpool.tile([P, C], BF16, tag="scratch")
                for q in range(32):
                    if q < N_ACT_PLAIN:
                        nc.scalar.activation(out=scratch[:], in_=mb[:],
                                             func=ACT.Square, scale=1.0,
                                             bias=cneg[:, q:q + 1])
                        nc.scalar.activation(
                            out=ohm4[:, :, q, :], in_=scratch[:],
                            func=ACT.Relu,
                            scale=cneg[:, N_ACT_PLAIN:N_ACT_PLAIN + 1],
                            bias=1.0)
                    else:
                        nc.vector.tensor_scalar(out=ohm4[:, :, q, :],
                                                in0=mb[:], scalar1=float(q),
                                                scalar2=None, op0=ALU.is_equal)
                for q in range(32):
                    nc.vector.tensor_scalar(out=ohr4[:, :, q, :], in0=rhb[:],
                                            scalar1=float(q), scalar2=None,
                                            op0=ALU.is_equal)
                for cc in range(NI):
                    nc.tensor.matmul(
                        ps[ti][:],
                        lhsT=ohm[:, cc * 128:(cc + 1) * 128],
                        rhs=ohr[:, cc * 128:(cc + 1) * 128],
                        start=(ci == 0 and cc == 0),
                        stop=(ci == NCHUNK - 1 and cc == NI - 1),
                    )
                if ci == NCHUNK - 1:
                    hsb = dpool.tile([P, 128], F32, tag="hsb")
                    nc.vector.tensor_copy(out=hsb[:], in_=ps[ti][:])
                    nc.sync.dma_start(out[:, ti * 128:(ti + 1) * 128], hsb[:])

            # software pipeline: A(si+1) emitted before B(si)
            cur = phase_a(0)
            for si in range(len(chunks)):
                nxt = phase_a(si + 1) if si + 1 < len(chunks) else None
                phase_b(si, *cur)
                cur = nxt
    nc.compile()
    return nc


_KERNELS = {}


def _get_kernels():
    if "mm" not in _KERNELS:
        _KERNELS["mm"] = _build_minmax()
        _KERNELS["hist"] = _build_hist()
    return _KERNELS["mm"], _KERNELS["hist"]


def _shard(flat):
    """Split [TOTAL] -> per-core padded [P, KTOT] tiles + pad values."""
    tiles, pads = [], []
    for c in range(NCORES):
        s = flat[c * SHARD:(c + 1) * SHARD]
        v0 = s[0]
        t = np.concatenate([s, np.full(PADN, v0, s.dtype)]).reshape(P, KTOT)
        tiles.append(t)
        pads.append(v0)
    return tiles, pads


def _psum_to_hist(X):
    """[P, 128] f32 psum dump -> [1024] f64 histogram.

    psum cell (m*4+g, rh*4+g') holds group-g counts on the g==g' diagonal."""
    Y = X.astype(np.float64).reshape(32, 4, 32, 4)   # [m, g, rh, g']
    diag = Y[:, np.arange(4), :, np.arange(4)]       # [g, m, rh]
    cnt = diag.sum(axis=0)                           # [m, rh]
    return cnt.T.ravel()                             # j = 32*rh + m


def kernel(prediction, target):
    nc_mm, nc_hist = _get_kernels()
    p = np.ascontiguousarray(np.asarray(prediction, dtype=np.float32).ravel())
    t = np.ascontiguousarray(np.asarray(target, dtype=np.float32).ravel())
    p_tiles, p_pads = _shard(p)
    t_tiles, t_pads = _shard(t)
    core_ids = list(range(NCORES))

    in_maps = [{"pv": p_tiles[c], "tv": t_tiles[c]} for c in core_ids]
    res = run_bass_kernel_spmd(nc_mm, in_maps, core_ids).results
    mm = np.stack([r["mm"][0] for r in res])        # [8, 2] = (-min, max)
    lo = np.float32(-(mm[:, 0].max()))
    hi = np.float32(mm[:, 1].max())

    dx = np.float32((hi - lo) / np.float32(NX - 1))
    A = np.float32(np.float32(1.0) / dx)
    B = np.float32(np.float32(-lo * A) + np.float32(0.5))
    ab = np.stack([np.full(P, A, np.float32), np.full(P, B, np.float32)],
                  axis=1)

    in_maps = [{"pv": p_tiles[c], "tv": t_tiles[c], "ab": ab}
               for c in core_ids]
    res = run_bass_kernel_spmd(nc_hist, in_maps, core_ids).results

    hp = np.zeros(1024, np.float64)
    ht = np.zeros(1024, np.float64)
    for c in core_ids:
        X = res[c]["hist"]                          # [P, 256] f32
        hp += _psum_to_hist(X[:, 0:128])
        ht += _psum_to_hist(X[:, 128:256])
        jp = res[c]["jpad"][0]                      # [2] i32, device pad bins
        hp[min(max(int(jp[0]), 0), 1023)] -= PADN
        ht[min(max(int(jp[1]), 0), 1023)] -= PADN

    hp[NX - 1] += hp[NX:].sum()
    ht[NX - 1] += ht[NX:].sum()
    cnt_p = np.cumsum(hp[:NX])
    cnt_t = np.cumsum(ht[:NX])

    n = np.float64(TOTAL)
    diff = np.abs(cnt_p / n - cnt_t / n)
    y = diff * diff
    x = np.linspace(np.float64(lo), np.float64(hi), NX)
    dxs = x[1:] - x[:-1]
    out = np.sum(0.5 * (y[1:] + y[:-1]) * dxs)
    return np.float32(out)


# revision 17
# speedup vs baseline: 2.4834x; 1.0034x over previous
"""CRPS loss kernel for Trainium2, 8 NeuronCores (SPMD data-parallel).

reference semantics:
    p, t = prediction.ravel(), target.ravel()       # N = 16,611,840 each
    lo, hi = min(min p, min t), max(max p, max t)
    x = linspace(lo, hi, 1000)  (f32)
    cdf_q(x_i) = #{v in q : v <= x_i} / N
    return trapz(|cdf_p - cdf_t|^2, x)

Device work (per core, 1/8 shard of each tensor):
  kernel A: min/max reduce (DVE X-reduces + Pool whole-tile max partials).
  kernel B: per element j = rint(v*A + B) in [0, 1000] (Act affine+round);
            digits m = j & 31 (DVE), rh = j >> 5 (Act scaled-round trick).
            Joint (m, rh) histogram via PACK4 block-diagonal PE matmuls:
            lhsT = one-hots of m (4 groups x 32 bins, column order m*4+g),
            rhs  = one-hots of rh (4 groups x 32 bins, order rh*4+g).
            PSUM accumulates the WHOLE tensor (counts < 2^24, exact f32);
            one psum->sbuf copy + DMA per tensor. The device's own binning
            of the pad value (partition 127 is all-pad) is exported as jpad
            so the host subtracts pads exactly.
Host: combine cores' [128, 256] f32 psum dumps -> exact 1024-bin histogram,
      fold j>=999, cumsum, 1000-point trapz in f64.
"""

import numpy as np
from concourse import bacc, mybir, tile
from concourse.bass_utils import run_bass_kernel_spmd

P = 128
NCORES = 8
TOTAL = 16 * 1 * 721 * 1440          # 16,611,840
SHARD = TOTAL // NCORES              # 2,076,480
KTOT = 16384                         # padded columns/core/tensor
PADN = P * KTOT - SHARD              # 20,672
NX = 1000
C = 512                              # chunk columns
NCHUNK = KTOT // C                   # 32 per tensor
NI = C // 4                          # PACK4 matmuls per chunk
RED = 4096                           # minmax reduce chunk

F32 = mybir.dt.float32
I32 = mybir.dt.int32
BF16 = mybir.dt.bfloat16
ALU = mybir.AluOpType
ACT = mybir.ActivationFunctionType

N_ACT_PLAIN = 6      # m-side bins built on Act (2-op square/relu)


def _build_minmax():
    nc = bacc.Bacc()
    ins = [
        nc.declare_dram_parameter("pv", [P, KTOT], F32, isOutput=False),
        nc.declare_dram_parameter("tv", [P, KTOT], F32, isOutput=False),
    ]
    out = nc.declare_dram_parameter("mm", [1, 2], F32, isOutput=True)

    with tile.TileContext(nc) as tc:
        with (
            tc.tile_pool(name="sbuf", bufs=6) as pool,
            tc.tile_pool(name="acc", bufs=1) as apool,
        ):
            ntile = 2 * (KTOT // RED)            # 8 tiles
            NPOOL = 5                            # tiles whose MAX runs on Pool
            mins = apool.tile([P, ntile], F32)
            maxs = apool.tile([P, ntile - NPOOL], F32)
            pmax1 = apool.tile([1, NPOOL], F32)
            t = 0
            for src in ins:
                for ci in range(KTOT // RED):
                    v = pool.tile([P, RED], F32, tag="v")
                    dmaeng = nc.sync if t % 2 == 0 else nc.scalar
                    dmaeng.dma_start(v[:], src[:, ci * RED:(ci + 1) * RED])
                    nc.vector.tensor_reduce(
                        mins[:, t:t + 1], v[:], mybir.AxisListType.X, ALU.min)
                    if t < NPOOL:
                        nc.gpsimd.tensor_reduce(
                            pmax1[:, t:t + 1], v[:], mybir.AxisListType.XYZWC,
                            ALU.max)
                    else:
                        nc.vector.tensor_reduce(
                            maxs[:, t - NPOOL:t - NPOOL + 1], v[:],
                            mybir.AxisListType.X, ALU.max)
                    t += 1
            pmin = apool.tile([P, 1], F32)
            pmax = apool.tile([P, 1], F32)
            nc.vector.tensor_reduce(pmin[:], mins[:], mybir.AxisListType.X,
                                    ALU.min)
            nc.vector.tensor_reduce(pmax[:], maxs[:], mybir.AxisListType.X,
                                    ALU.max)
            both = apool.tile([P, 2], F32)
            nc.vector.tensor_scalar(out=both[:, 0:1], in0=pmin[:], scalar1=-1.0,
                                    scalar2=None, op0=ALU.mult)
            nc.vector.tensor_copy(out=both[:, 1:2], in_=pmax[:])
            red = apool.tile([1, 2], F32)
            nc.gpsimd.tensor_reduce(red[:], both[:], mybir.AxisListType.C,
                                    ALU.max)
            pb = apool.tile([1, 1], F32)
            nc.vector.tensor_reduce(pb[:], pmax1[:], mybir.AxisListType.X,
                                    ALU.max)
            fin = apool.tile([1, 2], F32)
            nc.vector.tensor_copy(out=fin[:], in_=red[:])
            nc.vector.tensor_tensor(out=fin[:, 1:2], in0=red[:, 1:2],
                                    in1=pb[:], op=ALU.max)
            nc.sync.dma_start(out[:], fin[:])
    nc.compile()
    return nc


def _build_hist():
    nc = bacc.Bacc()
    ins = [
        nc.declare_dram_parameter("pv", [P, KTOT], F32, isOutput=False),
        nc.declare_dram_parameter("tv", [P, KTOT], F32, isOutput=False),
    ]
    ab_in = nc.declare_dram_parameter("ab", [P, 2], F32, isOutput=False)
    # raw psum dumps: [0:128] prediction, [128:256] target
    out = nc.declare_dram_parameter("hist", [P, 256], F32, isOutput=True)
    out_jp = nc.declare_dram_parameter("jpad", [1, 2], I32, isOutput=True)

    with tile.TileContext(nc) as tc:
        with (
            tc.tile_pool(name="data", bufs=3) as dpool,
            tc.tile_pool(name="dig", bufs=2) as gpool,
            tc.tile_pool(name="oh", bufs=2) as ohpool,
            tc.tile_pool(name="const", bufs=1) as cpool,
            tc.tile_pool(name="psum", bufs=1, space="PSUM") as pp,
        ):
            ab_raw = cpool.tile([P, 2], F32)
            nc.sync.dma_start(ab_raw[:], ab_in[:])
            ab_a = cpool.tile([P, 2], F32)
            nc.scalar.copy(out=ab_a[:], in_=ab_raw[:])
            # consts: rh-extract scale/bias, Act-build -q biases, -1.0 scale
            c32 = cpool.tile([P, 2], F32)
            nc.vector.memset(c32[:, 0:1], 0.03125)
            nc.vector.memset(c32[:, 1:2], -0.484375)
            cneg = cpool.tile([P, N_ACT_PLAIN + 1], F32)
            for k in range(N_ACT_PLAIN):
                nc.vector.memset(cneg[:, k:k + 1], -float(k))
            nc.vector.memset(cneg[:, N_ACT_PLAIN:N_ACT_PLAIN + 1], -1.0)

            ps_p = pp.tile([P, 128], F32, tag="psP")
            ps_t = pp.tile([P, 128], F32, tag="psT")
            ps = [ps_p, ps_t]

            chunks = [(ti, ci) for ti in range(2) for ci in range(NCHUNK)]

            def phase_a(si):
                ti, ci = chunks[si]
                v = dpool.tile([P, C], F32, tag="v")
                nc.sync.dma_start(v[:], ins[ti][:, ci * C:(ci + 1) * C])
                ji = gpool.tile([P, C], I32, tag="ji")
                nc.scalar.activation(out=ji[:], in_=v[:], func=ACT.Identity,
                                     scale=ab_a[:, 0:1], bias=ab_a[:, 1:2])
                if ci == 0:
                    # partition 127 is all padding: export device pad bin
                    nc.sync.dma_start(out_jp[:, ti:ti + 1],
                                      ji[127:128, 0:1])
                rh32 = gpool.tile([P, C], I32, tag="rh32")
                nc.scalar.activation(out=rh32[:], in_=ji[:], func=ACT.Identity,
                                     scale=c32[:, 0:1], bias=c32[:, 1:2])
                m32 = gpool.tile([P, C], I32, tag="m32")
                nc.vector.tensor_scalar(out=m32[:], in0=ji[:], scalar1=31,
                                        scalar2=None, op0=ALU.bitwise_and)
                mb = gpool.tile([P, C], BF16, tag="mb")
                nc.scalar.copy(out=mb[:], in_=m32[:])
                rhb = gpool.tile([P, C], BF16, tag="rhb")
                nc.gpsimd.tensor_copy(out=rhb[:], in_=rh32[:])
                return mb, rhb

            def phase_b(si, mb, rhb):
                ti, ci = chunks[si]
                ohm = ohpool.tile([P, 32 * C], BF16, tag="ohm")
                ohr = ohpool.tile([P, 32 * C], BF16, tag="ohr")
                ohm4 = ohm[:].rearrange("p (cc q g) -> p cc q g", q=32, g=4)
                ohr4 = ohr[:].rearrange("p (cc q g) -> p cc q g", q=32, g=4)
                scratch = gpool.tile([P, C], BF16, tag="scratch")
                for q in range(32):
                    if q < N_ACT_PLAIN:
                        nc.scalar.activation(out=scratch[:], in_=mb[:],
                                             func=ACT.Square, scale=1.0,
                                             bias=cneg[:, q:q + 1])
                        nc.scalar.activation(
                            out=ohm4[:, :, q, :], in_=scratch[:],
                            func=ACT.Relu,
                            scale=cneg[:, N_ACT_PLAIN:N_ACT_PLAIN + 1],
                            bias=1.0)
                    else:
                        nc.vector.tensor_scalar(out=ohm4[:, :, q, :],
                                                in0=mb[:], scalar1=float(q),
                                                scalar2=None, op0=ALU.is_equal)
                for q in range(32):
                    nc.vector.tensor_scalar(out=ohr4[:, :, q, :], in0=rhb[:],
                                            scalar1=float(q), scalar2=None,
                                            op0=ALU.is_equal)
                for cc in range(NI):
                    nc.tensor.matmul(
                        ps[ti][:],
                        lhsT=ohm[:, cc * 128:(cc + 1) * 128],
                        rhs=ohr[:, cc * 128:(cc + 1) * 128],
                        start=(ci == 0 and cc == 0),
                        stop=(ci == NCHUNK - 1 and cc == NI - 1),
                    )
                if ci == NCHUNK - 1:
                    hsb = dpool.tile([P, 128], F32, tag="hsb")
                    nc.vector.tensor_copy(out=hsb[:], in_=ps[ti][:])
                    nc.sync.dma_start(out[:, ti * 128:(ti + 1) * 128], hsb[:])

            # software pipeline: A(si+1) emitted before B(si)
            cur = phase_a(0)
            for si in range(len(chunks)):
                nxt = phase_a(si + 1) if si + 1 < len(chunks) else None
                phase_b(si, *cur)
                cur = nxt
    nc.compile()
    return nc


_KERNELS = {}


def _get_kernels():
    if "mm" not in _KERNELS:
        _KERNELS["mm"] = _build_minmax()
        _KERNELS["hist"] = _build_hist()
    return _KERNELS["mm"], _KERNELS["hist"]


def _shard(flat):
    """Split [TOTAL] -> per-core padded [P, KTOT] tiles + pad values."""
    tiles, pads = [], []
    for c in range(NCORES):
        s = flat[c * SHARD:(c + 1) * SHARD]
        v0 = s[0]
        t = np.concatenate([s, np.full(PADN, v0, s.dtype)]).reshape(P, KTOT)
        tiles.append(t)
        pads.append(v0)
    return tiles, pads


def _psum_to_hist(X):
    """[P, 128] f32 psum dump -> [1024] f64 histogram.

    psum cell (m*4+g, rh*4+g') holds group-g counts on the g==g' diagonal."""
    Y = X.astype(np.float64).reshape(32, 4, 32, 4)   # [m, g, rh, g']
    diag = Y[:, np.arange(4), :, np.arange(4)]       # [g, m, rh]
    cnt = diag.sum(axis=0)                           # [m, rh]
    return cnt.T.ravel()                             # j = 32*rh + m


def kernel(prediction, target):
    nc_mm, nc_hist = _get_kernels()
    p = np.ascontiguousarray(np.asarray(prediction, dtype=np.float32).ravel())
    t = np.ascontiguousarray(np.asarray(target, dtype=np.float32).ravel())
    p_tiles, p_pads = _shard(p)
    t_tiles, t_pads = _shard(t)
    core_ids = list(range(NCORES))

    in_maps = [{"pv": p_tiles[c], "tv": t_tiles[c]} for c in core_ids]
    res = run_bass_kernel_spmd(nc_mm, in_maps, core_ids).results
    mm = np.stack([r["mm"][0] for r in res])        # [8, 2] = (-min, max)
    lo = np.float32(-(mm[:, 0].max()))
    hi = np.float32(mm[:, 1].max())

    dx = np.float32((hi - lo) / np.float32(NX - 1))
    A = np.float32(np.float32(1.0) / dx)
    B = np.float32(np.float32(-lo * A) + np.float32(0.5))
    ab = np.stack([np.full(P, A, np.float32), np.full(P, B, np.float32)],
                  axis=1)

    in_maps = [{"pv": p_tiles[c], "tv": t_tiles[c], "ab": ab}
               for c in core_ids]
    res = run_bass_kernel_spmd(nc_hist, in_maps, core_ids).results

    hp = np.zeros(1024, np.float64)
    ht = np.zeros(1024, np.float64)
    for c in core_ids:
        X = res[c]["hist"]                          # [P, 256] f32
        hp += _psum_to_hist(X[:, 0:128])
        ht += _psum_to_hist(X[:, 128:256])
        jp = res[c]["jpad"][0]                      # [2] i32, device pad bins
        hp[min(max(int(jp[0]), 0), 1023)] -= PADN
        ht[min(max(int(jp[1]), 0), 1023)] -= PADN

    hp[NX - 1] += hp[NX:].sum()
    ht[NX - 1] += ht[NX:].sum()
    cnt_p = np.cumsum(hp[:NX])
    cnt_t = np.cumsum(ht[:NX])

    n = np.float64(TOTAL)
    diff = np.abs(cnt_p / n - cnt_t / n)
    y = diff * diff
    x = np.linspace(np.float64(lo), np.float64(hi), NX)
    dxs = x[1:] - x[:-1]
    out = np.sum(0.5 * (y[1:] + y[:-1]) * dxs)
    return np.float32(out)


# revision 18
# speedup vs baseline: 2.8207x; 1.1358x over previous
"""CRPS loss kernel for Trainium2, 8 NeuronCores (SPMD data-parallel).

reference semantics:
    p, t = prediction.ravel(), target.ravel()       # N = 16,611,840 each
    lo, hi = min(min p, min t), max(max p, max t)
    x = linspace(lo, hi, 1000)  (f32)
    cdf_q(x_i) = #{v in q : v <= x_i} / N
    return trapz(|cdf_p - cdf_t|^2, x)

Device work (per core, 1/8 shard of each tensor):
  kernel A: min/max reduce (DVE X-reduces + Pool whole-tile max partials).
  kernel B: per element j = rint(v*A + B) in [0, 1000] (Act affine+round);
            digits m = j & 31 (DVE), rh = j >> 5 (Act scaled-round trick).
            Joint (m, rh) histogram via PACK4 block-diagonal PE matmuls:
            lhsT = one-hots of m (4 groups x 32 bins, column order m*4+g),
            rhs  = one-hots of rh (4 groups x 32 bins, order rh*4+g).
            PSUM accumulates the WHOLE tensor (counts < 2^24, exact f32);
            one psum->sbuf copy + DMA per tensor. The device's own binning
            of the pad value (partition 127 is all-pad) is exported as jpad
            so the host subtracts pads exactly.
Host: combine cores' [128, 256] f32 psum dumps -> exact 1024-bin histogram,
      fold j>=999, cumsum, 1000-point trapz in f64.
"""

import numpy as np
from concourse import bacc, mybir, tile
from concourse.bass_utils import run_bass_kernel_spmd

P = 128
NCORES = 8
TOTAL = 16 * 1 * 721 * 1440          # 16,611,840
SHARD = TOTAL // NCORES              # 2,076,480
KTOT = 16384                         # padded columns/core/tensor
PADN = P * KTOT - SHARD              # 20,672
NX = 1000
C = 512                              # chunk columns
NCHUNK = KTOT // C                   # 32 per tensor
NI = C // 4                          # PACK4 matmuls per chunk
RED = 4096                           # minmax reduce chunk

F32 = mybir.dt.float32
I32 = mybir.dt.int32
BF16 = mybir.dt.bfloat16
ALU = mybir.AluOpType
ACT = mybir.ActivationFunctionType

N_ACT_PLAIN = 6      # m-side bins built on Act (2-op square/relu)


def _build_minmax():
    nc = bacc.Bacc()
    ins = [
        nc.declare_dram_parameter("pv", [P, KTOT], F32, isOutput=False),
        nc.declare_dram_parameter("tv", [P, KTOT], F32, isOutput=False),
    ]
    out = nc.declare_dram_parameter("mm", [1, 2], F32, isOutput=True)

    with tile.TileContext(nc) as tc:
        with (
            tc.tile_pool(name="sbuf", bufs=6) as pool,
            tc.tile_pool(name="acc", bufs=1) as apool,
        ):
            ntile = 2 * (KTOT // RED)            # 8 tiles
            NPOOL = 5                            # tiles whose MAX runs on Pool
            mins = apool.tile([P, ntile], F32)
            maxs = apool.tile([P, ntile - NPOOL], F32)
            pmax1 = apool.tile([1, NPOOL], F32)
            t = 0
            for src in ins:
                for ci in range(KTOT // RED):
                    v = pool.tile([P, RED], F32, tag="v")
                    dmaeng = nc.sync if t % 2 == 0 else nc.scalar
                    dmaeng.dma_start(v[:], src[:, ci * RED:(ci + 1) * RED])
                    nc.vector.tensor_reduce(
                        mins[:, t:t + 1], v[:], mybir.AxisListType.X, ALU.min)
                    if t < NPOOL:
                        nc.gpsimd.tensor_reduce(
                            pmax1[:, t:t + 1], v[:], mybir.AxisListType.XYZWC,
                            ALU.max)
                    else:
                        nc.vector.tensor_reduce(
                            maxs[:, t - NPOOL:t - NPOOL + 1], v[:],
                            mybir.AxisListType.X, ALU.max)
                    t += 1
            pmin = apool.tile([P, 1], F32)
            pmax = apool.tile([P, 1], F32)
            nc.vector.tensor_reduce(pmin[:], mins[:], mybir.AxisListType.X,
                                    ALU.min)
            nc.vector.tensor_reduce(pmax[:], maxs[:], mybir.AxisListType.X,
                                    ALU.max)
            both = apool.tile([P, 2], F32)
            nc.vector.tensor_scalar(out=both[:, 0:1], in0=pmin[:], scalar1=-1.0,
                                    scalar2=None, op0=ALU.mult)
            nc.vector.tensor_copy(out=both[:, 1:2], in_=pmax[:])
            red = apool.tile([1, 2], F32)
            nc.gpsimd.tensor_reduce(red[:], both[:], mybir.AxisListType.C,
                                    ALU.max)
            pb = apool.tile([1, 1], F32)
            nc.vector.tensor_reduce(pb[:], pmax1[:], mybir.AxisListType.X,
                                    ALU.max)
            fin = apool.tile([1, 2], F32)
            nc.vector.tensor_copy(out=fin[:], in_=red[:])
            nc.vector.tensor_tensor(out=fin[:, 1:2], in0=red[:, 1:2],
                                    in1=pb[:], op=ALU.max)
            nc.sync.dma_start(out[:], fin[:])
    nc.compile()
    return nc


def _build_hist():
    nc = bacc.Bacc()
    ins = [
        nc.declare_dram_parameter("pv", [P, KTOT], F32, isOutput=False),
        nc.declare_dram_parameter("tv", [P, KTOT], F32, isOutput=False),
    ]
    ab_in = nc.declare_dram_parameter("ab", [P, 2], F32, isOutput=False)
    # raw psum dumps: [0:128] prediction, [128:256] target
    out = nc.declare_dram_parameter("hist", [P, 256], F32, isOutput=True)
    out_jp = nc.declare_dram_parameter("jpad", [1, 2], I32, isOutput=True)

    with tile.TileContext(nc) as tc:
        with (
            tc.tile_pool(name="data", bufs=3) as dpool,
            tc.tile_pool(name="dig", bufs=2) as gpool,
            tc.tile_pool(name="oh", bufs=2) as ohpool,
            tc.tile_pool(name="const", bufs=1) as cpool,
            tc.tile_pool(name="psum", bufs=1, space="PSUM") as pp,
        ):
            ab_raw = cpool.tile([P, 2], F32)
            nc.sync.dma_start(ab_raw[:], ab_in[:])
            ab_a = cpool.tile([P, 2], F32)
            nc.scalar.copy(out=ab_a[:], in_=ab_raw[:])
            # consts: rh-extract scale/bias, Act-build -q biases, -1.0 scale
            c32 = cpool.tile([P, 2], F32)
            nc.vector.memset(c32[:, 0:1], 0.03125)
            nc.vector.memset(c32[:, 1:2], -0.484375)
            cneg = cpool.tile([P, N_ACT_PLAIN + 1], F32)
            for k in range(N_ACT_PLAIN):
                nc.vector.memset(cneg[:, k:k + 1], -float(k))
            nc.vector.memset(cneg[:, N_ACT_PLAIN:N_ACT_PLAIN + 1], -1.0)

            ps_p = pp.tile([P, 128], F32, tag="psP")
            ps_t = pp.tile([P, 128], F32, tag="psT")
            ps = [ps_p, ps_t]

            chunks = [(ti, ci) for ti in range(2) for ci in range(NCHUNK)]

            def phase_a(si):
                ti, ci = chunks[si]
                v = dpool.tile([P, C], F32, tag="v")
                nc.sync.dma_start(v[:], ins[ti][:, ci * C:(ci + 1) * C])
                ji = gpool.tile([P, C], I32, tag="ji")
                nc.scalar.activation(out=ji[:], in_=v[:], func=ACT.Identity,
                                     scale=ab_a[:, 0:1], bias=ab_a[:, 1:2])
                if ci == 0:
                    # partition 127 is all padding: export device pad bin
                    nc.sync.dma_start(out_jp[:, ti:ti + 1],
                                      ji[127:128, 0:1])
                rh32 = gpool.tile([P, C], I32, tag="rh32")
                nc.scalar.activation(out=rh32[:], in_=ji[:], func=ACT.Identity,
                                     scale=c32[:, 0:1], bias=c32[:, 1:2])
                m32 = gpool.tile([P, C], I32, tag="m32")
                nc.vector.tensor_scalar(out=m32[:], in0=ji[:], scalar1=31,
                                        scalar2=None, op0=ALU.bitwise_and)
                mb = gpool.tile([P, C], BF16, tag="mb")
                nc.scalar.copy(out=mb[:], in_=m32[:])
                rhb = gpool.tile([P, C], BF16, tag="rhb")
                nc.scalar.copy(out=rhb[:], in_=rh32[:])
                return mb, rhb

            def phase_b(si, mb, rhb):
                ti, ci = chunks[si]
                ohm = ohpool.tile([P, 32 * C], BF16, tag="ohm")
                ohr = ohpool.tile([P, 32 * C], BF16, tag="ohr")
                ohm4 = ohm[:].rearrange("p (cc q g) -> p cc q g", q=32, g=4)
                ohr4 = ohr[:].rearrange("p (cc q g) -> p cc q g", q=32, g=4)
                scratch = gpool.tile([P, C], BF16, tag="scratch")
                for q in range(32):
                    if q < N_ACT_PLAIN:
                        nc.scalar.activation(out=scratch[:], in_=mb[:],
                                             func=ACT.Square, scale=1.0,
                                             bias=cneg[:, q:q + 1])
                        nc.scalar.activation(
                            out=ohm4[:, :, q, :], in_=scratch[:],
                            func=ACT.Relu,
                            scale=cneg[:, N_ACT_PLAIN:N_ACT_PLAIN + 1],
                            bias=1.0)
                    else:
                        nc.vector.tensor_scalar(out=ohm4[:, :, q, :],
                                                in0=mb[:], scalar1=float(q),
                                                scalar2=None, op0=ALU.is_equal)
                for q in range(32):
                    nc.vector.tensor_scalar(out=ohr4[:, :, q, :], in0=rhb[:],
                                            scalar1=float(q), scalar2=None,
                                            op0=ALU.is_equal)
                for cc in range(NI):
                    nc.tensor.matmul(
                        ps[ti][:],
                        lhsT=ohm[:, cc * 128:(cc + 1) * 128],
                        rhs=ohr[:, cc * 128:(cc + 1) * 128],
                        start=(ci == 0 and cc == 0),
                        stop=(ci == NCHUNK - 1 and cc == NI - 1),
                    )
                if ci == NCHUNK - 1:
                    hsb = dpool.tile([P, 128], F32, tag="hsb")
                    nc.vector.tensor_copy(out=hsb[:], in_=ps[ti][:])
                    nc.sync.dma_start(out[:, ti * 128:(ti + 1) * 128], hsb[:])

            # software pipeline: A(si+1) emitted before B(si)
            cur = phase_a(0)
            for si in range(len(chunks)):
                nxt = phase_a(si + 1) if si + 1 < len(chunks) else None
                phase_b(si, *cur)
                cur = nxt
    nc.compile()
    return nc


_KERNELS = {}


def _get_kernels():
    if "mm" not in _KERNELS:
        _KERNELS["mm"] = _build_minmax()
        _KERNELS["hist"] = _build_hist()
    return _KERNELS["mm"], _KERNELS["hist"]


def _shard(flat):
    """Split [TOTAL] -> per-core padded [P, KTOT] tiles + pad values."""
    tiles, pads = [], []
    for c in range(NCORES):
        s = flat[c * SHARD:(c + 1) * SHARD]
        v0 = s[0]
        t = np.concatenate([s, np.full(PADN, v0, s.dtype)]).reshape(P, KTOT)
        tiles.append(t)
        pads.append(v0)
    return tiles, pads


def _psum_to_hist(X):
    """[P, 128] f32 psum dump -> [1024] f64 histogram.

    psum cell (m*4+g, rh*4+g') holds group-g counts on the g==g' diagonal."""
    Y = X.astype(np.float64).reshape(32, 4, 32, 4)   # [m, g, rh, g']
    diag = Y[:, np.arange(4), :, np.arange(4)]       # [g, m, rh]
    cnt = diag.sum(axis=0)                           # [m, rh]
    return cnt.T.ravel()                             # j = 32*rh + m


def kernel(prediction, target):
    nc_mm, nc_hist = _get_kernels()
    p = np.ascontiguousarray(np.asarray(prediction, dtype=np.float32).ravel())
    t = np.ascontiguousarray(np.asarray(target, dtype=np.float32).ravel())
    p_tiles, p_pads = _shard(p)
    t_tiles, t_pads = _shard(t)
    core_ids = list(range(NCORES))

    in_maps = [{"pv": p_tiles[c], "tv": t_tiles[c]} for c in core_ids]
    res = run_bass_kernel_spmd(nc_mm, in_maps, core_ids).results
    mm = np.stack([r["mm"][0] for r in res])        # [8, 2] = (-min, max)
    lo = np.float32(-(mm[:, 0].max()))
    hi = np.float32(mm[:, 1].max())

    dx = np.float32((hi - lo) / np.float32(NX - 1))
    A = np.float32(np.float32(1.0) / dx)
    B = np.float32(np.float32(-lo * A) + np.float32(0.5))
    ab = np.stack([np.full(P, A, np.float32), np.full(P, B, np.float32)],
                  axis=1)

    in_maps = [{"pv": p_tiles[c], "tv": t_tiles[c], "ab": ab}
               for c in core_ids]
    res = run_bass_kernel_spmd(nc_hist, in_maps, core_ids).results

    hp = np.zeros(1024, np.float64)
    ht = np.zeros(1024, np.float64)
    for c in core_ids:
        X = res[c]["hist"]                          # [P, 256] f32
        hp += _psum_to_hist(X[:, 0:128])
        ht += _psum_to_hist(X[:, 128:256])
        jp = res[c]["jpad"][0]                      # [2] i32, device pad bins
        hp[min(max(int(jp[0]), 0), 1023)] -= PADN
        ht[min(max(int(jp[1]), 0), 1023)] -= PADN

    hp[NX - 1] += hp[NX:].sum()
    ht[NX - 1] += ht[NX:].sum()
    cnt_p = np.cumsum(hp[:NX])
    cnt_t = np.cumsum(ht[:NX])

    n = np.float64(TOTAL)
    diff = np.abs(cnt_p / n - cnt_t / n)
    y = diff * diff
    x = np.linspace(np.float64(lo), np.float64(hi), NX)
    dxs = x[1:] - x[:-1]
    out = np.sum(0.5 * (y[1:] + y[:-1]) * dxs)
    return np.float32(out)


# revision 19
# speedup vs baseline: 2.9436x; 1.0436x over previous
"""CRPS loss kernel for Trainium2, 8 NeuronCores (SPMD data-parallel).

reference semantics:
    p, t = prediction.ravel(), target.ravel()       # N = 16,611,840 each
    lo, hi = min(min p, min t), max(max p, max t)
    x = linspace(lo, hi, 1000)  (f32)
    cdf_q(x_i) = #{v in q : v <= x_i} / N
    return trapz(|cdf_p - cdf_t|^2, x)

Device work (per core, 1/8 shard of each tensor):
  kernel A: min/max reduce (DVE X-reduces + Pool whole-tile max partials).
  kernel B: per element j = rint(v*A + B) in [0, 1000] (Act affine+round);
            digits m = j & 31 (DVE), rh = j >> 5 (Act scaled-round trick).
            Joint (m, rh) histogram via PACK4 block-diagonal PE matmuls:
            lhsT = one-hots of m (4 groups x 32 bins, column order m*4+g),
            rhs  = one-hots of rh (4 groups x 32 bins, order rh*4+g).
            PSUM accumulates the WHOLE tensor (counts < 2^24, exact f32);
            one psum->sbuf copy + DMA per tensor. The device's own binning
            of the pad value (partition 127 is all-pad) is exported as jpad
            so the host subtracts pads exactly.
Host: combine cores' [128, 256] f32 psum dumps -> exact 1024-bin histogram,
      fold j>=999, cumsum, 1000-point trapz in f64.
"""

import numpy as np
from concourse import bacc, mybir, tile
from concourse.bass_utils import run_bass_kernel_spmd

P = 128
NCORES = 8
TOTAL = 16 * 1 * 721 * 1440          # 16,611,840
SHARD = TOTAL // NCORES              # 2,076,480
KTOT = 16640                         # padded columns/core/tensor
PADN = P * KTOT - SHARD              # 53,440
NX = 1000
C = 640                              # chunk columns
NCHUNK = KTOT // C                   # 26 per tensor
NI = C // 4                          # PACK4 matmuls per chunk
RED = KTOT // 4                      # minmax reduce chunk

F32 = mybir.dt.float32
I32 = mybir.dt.int32
BF16 = mybir.dt.bfloat16
ALU = mybir.AluOpType
ACT = mybir.ActivationFunctionType

N_ACT_PLAIN = 6      # m-side bins built on Act (2-op square/relu)


def _build_minmax():
    nc = bacc.Bacc()
    ins = [
        nc.declare_dram_parameter("pv", [P, KTOT], F32, isOutput=False),
        nc.declare_dram_parameter("tv", [P, KTOT], F32, isOutput=False),
    ]
    out = nc.declare_dram_parameter("mm", [1, 2], F32, isOutput=True)

    with tile.TileContext(nc) as tc:
        with (
            tc.tile_pool(name="sbuf", bufs=6) as pool,
            tc.tile_pool(name="acc", bufs=1) as apool,
        ):
            ntile = 2 * (KTOT // RED)            # 8 tiles
            NPOOL = 5                            # tiles whose MAX runs on Pool
            mins = apool.tile([P, ntile], F32)
            maxs = apool.tile([P, ntile - NPOOL], F32)
            pmax1 = apool.tile([1, NPOOL], F32)
            t = 0
            for src in ins:
                for ci in range(KTOT // RED):
                    v = pool.tile([P, RED], F32, tag="v")
                    dmaeng = nc.sync if t % 2 == 0 else nc.scalar
                    dmaeng.dma_start(v[:], src[:, ci * RED:(ci + 1) * RED])
                    nc.vector.tensor_reduce(
                        mins[:, t:t + 1], v[:], mybir.AxisListType.X, ALU.min)
                    if t < NPOOL:
                        nc.gpsimd.tensor_reduce(
                            pmax1[:, t:t + 1], v[:], mybir.AxisListType.XYZWC,
                            ALU.max)
                    else:
                        nc.vector.tensor_reduce(
                            maxs[:, t - NPOOL:t - NPOOL + 1], v[:],
                            mybir.AxisListType.X, ALU.max)
                    t += 1
            pmin = apool.tile([P, 1], F32)
            pmax = apool.tile([P, 1], F32)
            nc.vector.tensor_reduce(pmin[:], mins[:], mybir.AxisListType.X,
                                    ALU.min)
            nc.vector.tensor_reduce(pmax[:], maxs[:], mybir.AxisListType.X,
                                    ALU.max)
            both = apool.tile([P, 2], F32)
            nc.vector.tensor_scalar(out=both[:, 0:1], in0=pmin[:], scalar1=-1.0,
                                    scalar2=None, op0=ALU.mult)
            nc.vector.tensor_copy(out=both[:, 1:2], in_=pmax[:])
            red = apool.tile([1, 2], F32)
            nc.gpsimd.tensor_reduce(red[:], both[:], mybir.AxisListType.C,
                                    ALU.max)
            pb = apool.tile([1, 1], F32)
            nc.vector.tensor_reduce(pb[:], pmax1[:], mybir.AxisListType.X,
                                    ALU.max)
            fin = apool.tile([1, 2], F32)
            nc.vector.tensor_copy(out=fin[:], in_=red[:])
            nc.vector.tensor_tensor(out=fin[:, 1:2], in0=red[:, 1:2],
                                    in1=pb[:], op=ALU.max)
            nc.sync.dma_start(out[:], fin[:])
    nc.compile()
    return nc


def _build_hist():
    nc = bacc.Bacc()
    ins = [
        nc.declare_dram_parameter("pv", [P, KTOT], F32, isOutput=False),
        nc.declare_dram_parameter("tv", [P, KTOT], F32, isOutput=False),
    ]
    ab_in = nc.declare_dram_parameter("ab", [P, 2], F32, isOutput=False)
    # raw psum dumps: [0:128] prediction, [128:256] target
    out = nc.declare_dram_parameter("hist", [P, 256], F32, isOutput=True)
    out_jp = nc.declare_dram_parameter("jpad", [1, 2], I32, isOutput=True)

    with tile.TileContext(nc) as tc:
        with (
            tc.tile_pool(name="data", bufs=3) as dpool,
            tc.tile_pool(name="dig", bufs=2) as gpool,
            tc.tile_pool(name="oh", bufs=2) as ohpool,
            tc.tile_pool(name="const", bufs=1) as cpool,
            tc.tile_pool(name="psum", bufs=1, space="PSUM") as pp,
        ):
            ab_raw = cpool.tile([P, 2], F32)
            nc.sync.dma_start(ab_raw[:], ab_in[:])
            ab_a = cpool.tile([P, 2], F32)
            nc.scalar.copy(out=ab_a[:], in_=ab_raw[:])
            # consts: rh-extract scale/bias, Act-build -q biases, -1.0 scale
            c32 = cpool.tile([P, 2], F32)
            nc.vector.memset(c32[:, 0:1], 0.03125)
            nc.vector.memset(c32[:, 1:2], -0.484375)
            cneg = cpool.tile([P, N_ACT_PLAIN + 1], F32)
            for k in range(N_ACT_PLAIN):
                nc.vector.memset(cneg[:, k:k + 1], -float(k))
            nc.vector.memset(cneg[:, N_ACT_PLAIN:N_ACT_PLAIN + 1], -1.0)

            ps_p = pp.tile([P, 128], F32, tag="psP")
            ps_t = pp.tile([P, 128], F32, tag="psT")
            ps = [ps_p, ps_t]

            chunks = [(ti, ci) for ti in range(2) for ci in range(NCHUNK)]

            def phase_a(si):
                ti, ci = chunks[si]
                v = dpool.tile([P, C], F32, tag="v")
                nc.sync.dma_start(v[:], ins[ti][:, ci * C:(ci + 1) * C])
                ji = gpool.tile([P, C], I32, tag="ji")
                nc.scalar.activation(out=ji[:], in_=v[:], func=ACT.Identity,
                                     scale=ab_a[:, 0:1], bias=ab_a[:, 1:2])
                if ci == 0:
                    # partition 127 is all padding: export device pad bin
                    nc.sync.dma_start(out_jp[:, ti:ti + 1],
                                      ji[127:128, 0:1])
                rh32 = gpool.tile([P, C], I32, tag="rh32")
                nc.scalar.activation(out=rh32[:], in_=ji[:], func=ACT.Identity,
                                     scale=c32[:, 0:1], bias=c32[:, 1:2])
                m32 = gpool.tile([P, C], I32, tag="m32")
                nc.vector.tensor_scalar(out=m32[:], in0=ji[:], scalar1=31,
                                        scalar2=None, op0=ALU.bitwise_and)
                mb = gpool.tile([P, C], BF16, tag="mb")
                nc.scalar.copy(out=mb[:], in_=m32[:])
                rhb = gpool.tile([P, C], BF16, tag="rhb")
                nc.scalar.copy(out=rhb[:], in_=rh32[:])
                return mb, rhb

            def phase_b(si, mb, rhb):
                ti, ci = chunks[si]
                ohm = ohpool.tile([P, 32 * C], BF16, tag="ohm")
                ohr = ohpool.tile([P, 32 * C], BF16, tag="ohr")
                ohm4 = ohm[:].rearrange("p (cc q g) -> p cc q g", q=32, g=4)
                ohr4 = ohr[:].rearrange("p (cc q g) -> p cc q g", q=32, g=4)
                scratch = gpool.tile([P, C], BF16, tag="scratch")
                for q in range(32):
                    if q < N_ACT_PLAIN:
                        nc.scalar.activation(out=scratch[:], in_=mb[:],
                                             func=ACT.Square, scale=1.0,
                                             bias=cneg[:, q:q + 1])
                        nc.scalar.activation(
                            out=ohm4[:, :, q, :], in_=scratch[:],
                            func=ACT.Relu,
                            scale=cneg[:, N_ACT_PLAIN:N_ACT_PLAIN + 1],
                            bias=1.0)
                    else:
                        nc.vector.tensor_scalar(out=ohm4[:, :, q, :],
                                                in0=mb[:], scalar1=float(q),
                                                scalar2=None, op0=ALU.is_equal)
                for q in range(32):
                    nc.vector.tensor_scalar(out=ohr4[:, :, q, :], in0=rhb[:],
                                            scalar1=float(q), scalar2=None,
                                            op0=ALU.is_equal)
                for cc in range(NI):
                    nc.tensor.matmul(
                        ps[ti][:],
                        lhsT=ohm[:, cc * 128:(cc + 1) * 128],
                        rhs=ohr[:, cc * 128:(cc + 1) * 128],
                        start=(ci == 0 and cc == 0),
                        stop=(ci == NCHUNK - 1 and cc == NI - 1),
                    )
                if ci == NCHUNK - 1:
                    hsb = dpool.tile([P, 128], F32, tag="hsb")
                    nc.vector.tensor_copy(out=hsb[:], in_=ps[ti][:])
                    nc.sync.dma_start(out[:, ti * 128:(ti + 1) * 128], hsb[:])

            # software pipeline: A(si+1) emitted before B(si)
            cur = phase_a(0)
            for si in range(len(chunks)):
                nxt = phase_a(si + 1) if si + 1 < len(chunks) else None
                phase_b(si, *cur)
                cur = nxt
    nc.compile()
    return nc


_KERNELS = {}


def _get_kernels():
    if "mm" not in _KERNELS:
        _KERNELS["mm"] = _build_minmax()
        _KERNELS["hist"] = _build_hist()
    return _KERNELS["mm"], _KERNELS["hist"]


def _shard(flat):
    """Split [TOTAL] -> per-core padded [P, KTOT] tiles + pad values."""
    tiles, pads = [], []
    for c in range(NCORES):
        s = flat[c * SHARD:(c + 1) * SHARD]
        v0 = s[0]
        t = np.concatenate([s, np.full(PADN, v0, s.dtype)]).reshape(P, KTOT)
        tiles.append(t)
        pads.append(v0)
    return tiles, pads


def _psum_to_hist(X):
    """[P, 128] f32 psum dump -> [1024] f64 histogram.

    psum cell (m*4+g, rh*4+g') holds group-g counts on the g==g' diagonal."""
    Y = X.astype(np.float64).reshape(32, 4, 32, 4)   # [m, g, rh, g']
    diag = Y[:, np.arange(4), :, np.arange(4)]       # [g, m, rh]
    cnt = diag.sum(axis=0)                           # [m, rh]
    return cnt.T.ravel()                             # j = 32*rh + m


def kernel(prediction, target):
    nc_mm, nc_hist = _get_kernels()
    p = np.ascontiguousarray(np.asarray(prediction, dtype=np.float32).ravel())
    t = np.ascontiguousarray(np.asarray(target, dtype=np.float32).ravel())
    p_tiles, p_pads = _shard(p)
    t_tiles, t_pads = _shard(t)
    core_ids = list(range(NCORES))

    in_maps = [{"pv": p_tiles[c], "tv": t_tiles[c]} for c in core_ids]
    res = run_bass_kernel_spmd(nc_mm, in_maps, core_ids).results
    mm = np.stack([r["mm"][0] for r in res])        # [8, 2] = (-min, max)
    lo = np.float32(-(mm[:, 0].max()))
    hi = np.float32(mm[:, 1].max())

    dx = np.float32((hi - lo) / np.float32(NX - 1))
    A = np.float32(np.float32(1.0) / dx)
    B = np.float32(np.float32(-lo * A) + np.float32(0.5))
    ab = np.stack([np.full(P, A, np.float32), np.full(P, B, np.float32)],
                  axis=1)

    in_maps = [{"pv": p_tiles[c], "tv": t_tiles[c], "ab": ab}
               for c in core_ids]
    res = run_bass_kernel_spmd(nc_hist, in_maps, core_ids).results

    hp = np.zeros(1024, np.float64)
    ht = np.zeros(1024, np.float64)
    for c in core_ids:
        X = res[c]["hist"]                          # [P, 256] f32
        hp += _psum_to_hist(X[:, 0:128])
        ht += _psum_to_hist(X[:, 128:256])
        jp = res[c]["jpad"][0]                      # [2] i32, device pad bins
        hp[min(max(int(jp[0]), 0), 1023)] -= PADN
        ht[min(max(int(jp[1]), 0), 1023)] -= PADN

    hp[NX - 1] += hp[NX:].sum()
    ht[NX - 1] += ht[NX:].sum()
    cnt_p = np.cumsum(hp[:NX])
    cnt_t = np.cumsum(ht[:NX])

    n = np.float64(TOTAL)
    diff = np.abs(cnt_p / n - cnt_t / n)
    y = diff * diff
    x = np.linspace(np.float64(lo), np.float64(hi), NX)
    dxs = x[1:] - x[:-1]
    out = np.sum(0.5 * (y[1:] + y[:-1]) * dxs)
    return np.float32(out)
